# revision 1
# baseline (speedup 1.0000x reference)
"""Trainium2 Bass kernel for nn_CrossSlideConsistencyLoss.

Computes, for 3 slides of 8192 2-D points each:
  - radial histogram (20 bins) of centered radii
  - |FFT|[0:5] of the mean-centered angular histogram (72 bins)
  - collision rate: fraction of points whose nearest neighbor is < 0.01 away
then the mean over descriptor components of the across-slide variance (ddof=1).

Strategy (8 NeuronCores, SPMD):
  - Host sorts each slide's points by x (pure permutation; every descriptor
    piece is permutation invariant). Any pair closer than 0.01 is then within
    W ranks of each other (validated at runtime), so the NxN cdist collapses
    to a banded window per 128-row block:
      one K=5 matmul per block: z = 2 x_i x_j + 2 y_i y_j - sq_j,
      and d2 < th  <=>  z > sq_i - th, counted in one compare+accumulate pass
      (ACT sign+accum for some blocks, DVE is_gt+accum for the rest).
    Rows are sharded over the 8 cores (1024 rows/core, 24 blocks of
    [128 x WIN] per core across the 3 slides).
  - Angular DFT: only |FFT| bins 1..4 of the angular histogram are needed
    (bin 0 of the mean-centered histogram is ~0 and contributes ~0 variance);
    they equal direct sums of cos/sin(2 pi k aidx / 72) over points -
    computed shard-local on every core and summed by the final AllReduce.
  - Radial histogram needs the global max radius, so one core per slide bins
    it whole (mask-gated); the other cores contribute zeros.
  - One 512B AllReduce combines [radial counts | DFT sums | collision
    counts]; every core then computes the final variance; core 0's scalar
    output is returned.
"""
import numpy as np

import concourse.bass as bass
import concourse.bacc as bacc
import concourse.bass_isa as bass_isa
import concourse.mybir as mybir
import concourse.tile as tile
from concourse.bass_utils import run_bass_kernel_spmd

F32 = mybir.dt.float32
F32R = mybir.dt.float32r
Alu = mybir.AluOpType
Act = mybir.ActivationFunctionType

N = 8192
N_CORES = 8
NSLIDES = 3
ROWS_PER_CORE = N // N_CORES          # 1024
NBLK = ROWS_PER_CORE // 128           # 8 blocks per core per slide
SH = NSLIDES * NBLK                   # 24 shard columns
NDVE = 4                              # collision blocks per slide on DVE
TH = 1e-4                             # d^2 threshold (0.01^2)
PI = float(np.pi)
R2C = 12582912.0                      # 1.5 * 2^23: rne magic constant

# AllReduce vector layout ([1, 128] f32):
#   [0:63)   radial cumcounts, per slide: [C0=0, C1..C20] (21 cols x 3)
#   [63:87)  DFT sums: 12 cos (k-major, s-minor), then 12 sin
#   [87:90)  collision row counts per slide
V_DFT = 63
V_COLL = 87


def _bcast(ap, axis_len, at):
    """Insert a broadcast (stride-0) dim of length axis_len at free position
    `at` (0 = before the flattened free dim, 1 = after it)."""
    p, f = ap.shape[0], int(np.prod(ap.shape[1:]))
    if at == 0:
        return ap.rearrange("p (a b) -> p a b", a=1).to_broadcast([p, axis_len, f])
    return ap.rearrange("p (a b) -> p a b", b=1).to_broadcast([p, f, axis_len])


def build_program(win, collective=True):
    buf = ROWS_PER_CORE + win - 128   # rhs window buffer length per core/slide
    bufp = buf // 128                 # p-major free dim of the buffer
    assert buf % 128 == 0

    # mega-input column layout (two DMAs load everything)
    # m128 [128, F1]: pxy(3*128) | rxy(128) | pw(3*2*bufp) | xsh(24) | ysh(24)
    #                 | kvec(20) | k4(4) | mask row0 (64)
    O_PXY, O_RXY = 0, 384
    O_PW = O_RXY + 128
    O_XSH = O_PW + NSLIDES * 2 * bufp
    O_YSH = O_XSH + SH
    O_KVEC = O_YSH + SH
    O_K4 = O_KVEC + 20
    O_MASK = O_K4 + 4
    F1 = O_MASK + 64
    # m5 [5, F2]: rhs buffers (3*buf; row4 device-filled) | lhs (3*1024)
    O_LHS = NSLIDES * buf
    F2 = O_LHS + NSLIDES * ROWS_PER_CORE

    nc = bacc.Bacc("TRN2", target_bir_lowering=False, debug=False, num_devices=N_CORES)
    i_m128 = nc.dram_tensor("m128", [128, F1], F32, kind="ExternalInput")
    i_m5 = nc.dram_tensor("m5", [5, F2], F32R, kind="ExternalInput")
    o_out = nc.dram_tensor("out", [1, 1], F32, kind="ExternalOutput")

    with tile.TileContext(nc) as tc:
        with (
            tc.tile_pool(name="cst", bufs=1) as cst,
            tc.tile_pool(name="scr", bufs=3) as scp,
            tc.tile_pool(name="psum",
                         bufs=max(1, 6 // max(1, win * 4 // 2048)),
                         space="PSUM") as pp,
            tc.tile_pool(name="psv", bufs=1, space="PSUM") as pv,
            tc.tile_pool(name="dram", bufs=1, space="DRAM") as dr,
        ):
            # ---------------- input loads (2 DMAs) ----------------
            big128 = cst.tile([128, F1], F32, tag="big128")
            nc.sync.dma_start(big128[:, 0:O_XSH], i_m128[:, 0:O_XSH])
            nc.sync.dma_start(big128[:, O_XSH:F1], i_m128[:, O_XSH:F1])
            big5 = cst.tile([5, F2], F32R, tag="big5")
            nc.sync.dma_start(big5[:], i_m5[:])

            pxys = [big128[:, O_PXY + 128 * s:O_PXY + 128 * (s + 1)] for s in range(NSLIDES)]
            rxy = big128[:, O_RXY:O_RXY + 128]
            pws = [big128[:, O_PW + 2 * bufp * s:O_PW + 2 * bufp * (s + 1)] for s in range(NSLIDES)]
            xsh = big128[:, O_XSH:O_XSH + SH]
            ysh = big128[:, O_YSH:O_YSH + SH]
            kvecT = big128[:, O_KVEC:O_KVEC + 20]
            k4T = big128[:, O_K4:O_K4 + 4]
            maskR = big128[0:1, O_MASK:O_MASK + 64]
            rhs = [big5[:, buf * s:buf * (s + 1)] for s in range(NSLIDES)]
            lhsT = big5[:, O_LHS:O_LHS + NSLIDES * ROWS_PER_CORE]

            ones128 = cst.tile([128, 1], F32, tag="ones128")
            nc.gpsimd.memset(ones128[:], 1.0)
            invN128 = cst.tile([128, 1], F32, tag="invN128")
            nc.gpsimd.memset(invN128[:], 1.0 / N)
            nc.scalar.add_instruction(mybir.InstLoadActFuncSet(
                act_func_set_id=9, name=f"I-{nc.next_id()}", ins=[], outs=[]))

            # ---------------- centers (full slides + radial slide) ----------------
            c8p = pv.tile([1, 8], F32, tag="c8p")
            for s in range(NSLIDES):
                csum = scp.tile([128, 2], F32, tag="csum")
                nc.vector.tensor_reduce(
                    csum[:], pxys[s].rearrange("p (t f) -> p t f", t=2),
                    mybir.AxisListType.X, Alu.add,
                )
                nc.tensor.matmul(c8p[0:1, 2 * s:2 * s + 2], ones128[:], csum[:])
            rsum = scp.tile([128, 2], F32, tag="csum")
            nc.vector.tensor_reduce(
                rsum[:], rxy.rearrange("p (t f) -> p t f", t=2),
                mybir.AxisListType.X, Alu.add,
            )
            nc.tensor.matmul(c8p[0:1, 6:8], ones128[:], rsum[:])

            crowX = cst.tile([1, SH], F32, tag="crowX")
            nc.scalar.activation(
                crowX[:].rearrange("p (s b) -> p s b", s=NSLIDES),
                c8p[0:1, 0:6].rearrange("p (s t) -> p s t", s=NSLIDES)[:, :, 0:1]
                .to_broadcast([1, NSLIDES, NBLK]),
                Act.Copy, scale=1.0 / N,
            )
            crowY = cst.tile([1, SH], F32, tag="crowY")
            nc.scalar.activation(
                crowY[:].rearrange("p (s b) -> p s b", s=NSLIDES),
                c8p[0:1, 0:6].rearrange("p (s t) -> p s t", s=NSLIDES)[:, :, 1:2]
                .to_broadcast([1, NSLIDES, NBLK]),
                Act.Copy, scale=1.0 / N,
            )
            c2t = cst.tile([1, 2], F32, tag="c2t")
            nc.scalar.activation(c2t[:], c8p[0:1, 6:8], Act.Copy, scale=1.0 / N)
            cX24 = cst.tile([128, SH], F32, tag="cX24")
            nc.gpsimd.partition_broadcast(cX24[:], crowX[:])
            cY24 = cst.tile([128, SH], F32, tag="cY24")
            nc.gpsimd.partition_broadcast(cY24[:], crowY[:])
            crB = cst.tile([128, 2], F32, tag="crB")
            nc.gpsimd.partition_broadcast(crB[:], c2t[:])

            # ---------------- rhs row 4: sq_j over the window buffer ----------------
            for s in range(NSLIDES):
                xw = pws[s][:, 0:bufp]
                yw = pws[s][:, bufp:2 * bufp]
                sqw = cst.tile([128, bufp], F32, tag=f"sqw{s}")
                nc.gpsimd.tensor_tensor(sqw[:], xw, xw, Alu.mult)
                sqw2 = cst.tile([128, bufp], F32, tag=f"sqw2{s}")
                nc.gpsimd.tensor_tensor(sqw2[:], yw, yw, Alu.mult)
                nc.gpsimd.tensor_tensor(sqw[:], sqw[:], sqw2[:], Alu.add)
                sqwr = cst.tile([128, bufp], F32R, tag=f"sqwr{s}")
                nc.vector.tensor_copy(sqwr[:], sqw[:])
                # relayout [128, bufp] (p-major) -> [1, buf] sorted order
                nc.sync.dma_start(
                    rhs[s][4:5, :].rearrange("o (p f) -> o p f", p=128), sqwr[:]
                )

            # ------- shard prep on gpsimd (off the busy DVE/ACT engines) -------
            sqsh = cst.tile([128, SH], F32, tag="sqsh")
            nc.gpsimd.tensor_tensor(sqsh[:], xsh, xsh, Alu.mult)
            sqsh2 = scp.tile([128, SH], F32, tag="sqsh2")
            nc.gpsimd.tensor_tensor(sqsh2[:], ysh, ysh, Alu.mult)
            nc.gpsimd.tensor_tensor(sqsh[:], sqsh[:], sqsh2[:], Alu.add)
            biasA = cst.tile([128, SH], F32, tag="biasA")
            nc.gpsimd.tensor_scalar(biasA[:], sqsh[:], -1.0, TH, Alu.mult, Alu.add)
            thrD = cst.tile([128, SH], F32, tag="thrD")
            nc.gpsimd.tensor_scalar(thrD[:], sqsh[:], TH, None, Alu.subtract)

            # ---------------- r chains (both Sqrt ops cluster early) ----------------
            dx = scp.tile([128, SH], F32, tag="dx")
            nc.vector.tensor_tensor(dx[:], xsh, cX24[:], Alu.subtract)
            dy = scp.tile([128, SH], F32, tag="dy")
            nc.vector.tensor_tensor(dy[:], ysh, cY24[:], Alu.subtract)
            neg = scp.tile([128, SH], F32, tag="neg")
            nc.gpsimd.tensor_scalar(neg[:], dx[:], 0.0, None, Alu.is_lt)
            c1 = scp.tile([128, SH], F32, tag="c1")
            nc.gpsimd.tensor_scalar(c1[:], neg[:], -2.0, 1.0, Alu.mult, Alu.add)
            sy36 = scp.tile([128, SH], F32, tag="sy36")
            nc.gpsimd.tensor_scalar(sy36[:], dy[:], 0.0, 72.0, Alu.is_ge, Alu.mult)
            nc.gpsimd.tensor_scalar(sy36[:], sy36[:], 36.0, None, Alu.subtract)
            pn36p = scp.tile([128, SH], F32, tag="pn36p")
            nc.gpsimd.tensor_tensor(pn36p[:], neg[:], sy36[:], Alu.mult)
            pn36p_bi = nc.gpsimd.tensor_scalar(pn36p[:], pn36p[:], 36.0, None, Alu.add)
            r2 = scp.tile([128, SH], F32, tag="r2")
            nc.vector.tensor_tensor(r2[:], dx[:], dx[:], Alu.mult)
            yy = scp.tile([128, SH], F32, tag="yy")
            nc.vector.tensor_tensor(yy[:], dy[:], dy[:], Alu.mult)
            nc.vector.tensor_tensor(r2[:], r2[:], yy[:], Alu.add)
            I32 = mybir.dt.int32
            ih = scp.tile([128, SH], I32, tag="ih")
            nc.vector.tensor_scalar(ih[:], r2[:].bitcast(I32), 1, None, Alu.arith_shift_right)
            nc.vector.tensor_scalar(ih[:], ih[:], -1, 0x5F3759DF, Alu.mult, Alu.add)
            ny = scp.tile([128, SH], F32, tag="ny")
            nc.vector.tensor_copy(ny[:], ih[:].bitcast(F32))
            for _ in range(2):
                nt = scp.tile([128, SH], F32, tag="nt")
                nc.vector.tensor_tensor(nt[:], ny[:], ny[:], Alu.mult)
                nc.vector.scalar_tensor_tensor(nt[:], nt[:], -0.5, r2[:], Alu.mult, Alu.mult)
                nc.vector.scalar_tensor_tensor(ny[:], nt[:], 1.5, ny[:], Alu.add, Alu.mult)
            rr = scp.tile([128, SH], F32, tag="rr")
            nc.vector.tensor_tensor(rr[:], r2[:], ny[:], Alu.mult)

            dxf = scp.tile([128, 64], F32, tag="dxf")
            nc.gpsimd.tensor_scalar(dxf[:], rxy[:, 0:64], crB[:, 0:1], None, Alu.subtract)
            dyf = scp.tile([128, 64], F32, tag="dyf")
            nc.gpsimd.tensor_scalar(dyf[:], rxy[:, 64:128], crB[:, 1:2], None, Alu.subtract)
            rf2 = scp.tile([128, 64], F32, tag="rf2")
            nc.vector.tensor_tensor(rf2[:], dxf[:], dxf[:], Alu.mult)
            yyf = scp.tile([128, 64], F32, tag="yyf")
            nc.vector.tensor_tensor(yyf[:], dyf[:], dyf[:], Alu.mult)
            nc.vector.tensor_tensor(rf2[:], rf2[:], yyf[:], Alu.add)
            rmx = scp.tile([128, 1], F32, tag="rmx")
            nc.vector.tensor_reduce(rmx[:], rf2[:], mybir.AxisListType.X, Alu.max)
            rmxB = scp.tile([128, 1], F32, tag="rmxB")
            nc.gpsimd.partition_all_reduce(rmxB[:], rmx[:], 128, bass_isa.ReduceOp.max)
            rm1 = scp.tile([128, 1], F32, tag="rm1")
            nc.vector.tensor_scalar(rm1[:], rmxB[:], 1e-8, None, Alu.add)
            thrT = scp.tile([128, 20], F32, tag="thrT")
            nc.vector.tensor_scalar(thrT[:], kvecT, rm1[:, 0:1], None, Alu.mult)

            # ---------------- collision blocks ----------------
            acc = cst.tile([128, SH], F32, tag="acc")
            vecp = pv.tile([1, 64], F32, tag="vecp")
            sign_insts = []
            for s in range(NSLIDES):
                for b in range(NBLK):
                    col = s * NBLK + b
                    zp = pp.tile([128, win], F32, tag="zp")
                    for off in range(0, win, 512):  # moving free dim cap is 512
                        nc.tensor.matmul(
                            zp[:, off:off + min(512, win - off)],
                            lhsT[:, s * ROWS_PER_CORE + b * 128:
                                 s * ROWS_PER_CORE + b * 128 + 128],
                            rhs[s][:, b * 128 + off:b * 128 + off + min(512, win - off)],
                        )
                    if b < NBLK - (NDVE - (1 if s == 0 else 0)):
                        sg = scp.tile([128, win], F32, tag="sg")
                        sign_insts.append(nc.scalar.activation(
                            sg[:], zp[:], Act.Sign,
                            bias=biasA[:, col:col + 1], accum_out=acc[:, col:col + 1],
                        ).ins)
                    else:
                        sg = scp.tile([128, win], F32, tag="sgd")
                        nc.vector.tensor_scalar(
                            sg[:], zp[:], thrD[:, col:col + 1], None, Alu.is_gt,
                            Alu.add, accum_out=acc[:, col:col + 1],
                        )
            # ACT cols: sumsign > 3-win <=> count >= 2; DVE cols: count > 1.5
            ind = scp.tile([128, SH], F32, tag="ind")
            accv = acc[:].rearrange("p (s b) -> p s b", s=NSLIDES)
            indv = ind[:].rearrange("p (s b) -> p s b", s=NSLIDES)
            for s in range(NSLIDES):
                nA = NBLK - (NDVE - (1 if s == 0 else 0))
                nc.vector.tensor_scalar(indv[:, s:s + 1, 0:nA], accv[:, s:s + 1, 0:nA],
                                        float(3.0 - win), None, Alu.is_gt)
                nc.vector.tensor_scalar(indv[:, s:s + 1, nA:NBLK], accv[:, s:s + 1, nA:NBLK],
                                        1.5, None, Alu.is_gt)
            indR = scp.tile([128, NSLIDES], F32, tag="indR")
            nc.vector.tensor_reduce(
                indR[:], ind[:].rearrange("p (s b) -> p s b", s=NSLIDES),
                mybir.AxisListType.X, Alu.add,
            )
            nc.tensor.matmul(vecp[0:1, 44:47], invN128[:], indR[:])

            # ---------------- angular DFT over the core's shard ----------------
            adx = scp.tile([128, SH], F32, tag="adx")
            nc.scalar.activation(adx[:], dx[:], Act.Abs)
            den = scp.tile([128, SH], F32, tag="den")
            nc.vector.scalar_tensor_tensor(den[:], rr[:], 1e-38, adx[:], Alu.add, Alu.add)
            rden = scp.tile([128, SH], F32, tag="rden")
            nc.vector.reciprocal(rden[:], den[:])
            qt = scp.tile([128, SH], F32, tag="qt")
            nc.vector.tensor_tensor(qt[:], dy[:], rden[:], Alu.mult)
            at = scp.tile([128, SH], F32, tag="at")
            at_bi = nc.scalar.activation(at[:], qt[:], Act.Arctan)
            bass._add_dep_helper(at_bi.ins, sign_insts[3], False,
                                 "order: arctan after 4 collision signs")
            t1 = scp.tile([128, SH], F32, tag="t1")
            nc.vector.tensor_tensor(t1[:], at[:], c1[:], Alu.mult)
            ut = scp.tile([128, SH], F32, tag="ut")
            nc.vector.scalar_tensor_tensor(ut[:], t1[:], 72.0 / PI, pn36p[:], Alu.mult, Alu.add)
            # negative floor via rne trick: floor = rne - (rne > u); nfl = -floor
            rv = scp.tile([128, SH], F32, tag="rv")
            nc.vector.tensor_scalar(rv[:], ut[:], R2C, R2C, Alu.add, Alu.subtract)
            cmp = scp.tile([128, SH], F32, tag="cmp")
            nc.vector.tensor_tensor(cmp[:], rv[:], ut[:], Alu.is_gt)
            nfl = scp.tile([128, SH], F32, tag="nfl")
            nc.vector.tensor_tensor(nfl[:], cmp[:], rv[:], Alu.subtract)

            # ka = k * clip(-aidx); DFT magnitudes are sign-invariant
            ka = scp.tile([128, 4 * SH], F32, tag="ka")
            nc.vector.scalar_tensor_tensor(
                ka[:].rearrange("p (k f) -> p k f", k=4),
                _bcast(nfl[:], 4, 0), -71.0, _bcast(k4T, SH, 1), Alu.max, Alu.mult,
            )
            # range-reduce: sin(2*pi*v), v - rne(v) in [-0.5, 0.5] (1-periodic);
            # one fused [sin | cos] pass, cos(x) = sin(x + pi/2) via +0.25 turns
            sincos = cst.tile([128, 24], F32, tag="sincos")
            vb = scp.tile([128, 8 * SH], F32, tag="vb")
            nc.vector.tensor_scalar(vb[:, 0:4 * SH], ka[:], 1.0 / 72.0, None, Alu.mult)
            nc.vector.tensor_scalar(vb[:, 4 * SH:8 * SH], ka[:], 1.0 / 72.0, 0.25,
                                    Alu.mult, Alu.add)
            mb = scp.tile([128, 8 * SH], F32, tag="mb")
            nc.vector.tensor_scalar(mb[:], vb[:], R2C, R2C, Alu.add, Alu.subtract)
            nc.vector.tensor_tensor(mb[:], vb[:], mb[:], Alu.subtract)
            scv = scp.tile([128, 8 * SH], F32, tag="scv")
            sin_bi = nc.scalar.activation(scv[:], mb[:], Act.Sin, scale=2.0 * PI)
            bass._add_dep_helper(sin_bi.ins, sign_insts[7], False,
                                 "order: trig sin after 8 collision signs")
            nc.vector.tensor_reduce(
                sincos[:, 12:24].rearrange("p (k s) -> p k s", k=4),
                scv[:, 0:4 * SH].rearrange("p (k s b) -> p k s b", k=4, s=NSLIDES),
                mybir.AxisListType.X, Alu.add,
            )
            trig_red2_bi = nc.vector.tensor_reduce(
                sincos[:, 0:12].rearrange("p (k s) -> p k s", k=4),
                scv[:, 4 * SH:8 * SH].rearrange("p (k s b) -> p k s b", k=4, s=NSLIDES),
                mybir.AxisListType.X, Alu.add,
            )
            nc.tensor.matmul(vecp[0:1, 20:44], ones128[:], sincos[:])

            # ---------------- radial histogram of the owned slide ----------------
            ct = scp.tile([128, 20 * 64], F32, tag="ct")
            ct_bi = nc.vector.tensor_tensor(
                ct[:].rearrange("p (k f) -> p k f", k=20),
                _bcast(rf2[:], 20, 0), _bcast(thrT[:], 64, 1), Alu.is_lt,
            )
            bass._add_dep_helper(ct_bi.ins, trig_red2_bi.ins, False,
                                 "order: radial compare after angular trig reduces")
            cr = scp.tile([128, 20], F32, tag="cr")
            cr_bi = nc.vector.tensor_reduce(
                cr[:], ct[:].rearrange("p (k f) -> p k f", k=20),
                mybir.AxisListType.X, Alu.add,
            )
            bass._add_dep_helper(cr_bi.ins, trig_red2_bi.ins, False,
                                 "order: radial reduce after angular trig reduces")
            nc.tensor.matmul(vecp[0:1, 0:20], ones128[:], cr[:])

            # ---------------- assemble + AllReduce ----------------
            vecS = cst.tile([1, 128], F32, tag="vecS")
            nc.gpsimd.memset(vecS[:], 0.0)
            radview = vecS[0:1, 1:64].rearrange(
                "p (s k) -> p s k", s=NSLIDES, k=21
            )[:, :, 0:20]
            nc.vector.tensor_tensor(
                radview,
                vecp[0:1, 0:20].rearrange("p (o k) -> p o k", o=1)
                .to_broadcast([1, NSLIDES, 20]),
                maskR[0:1, 0:60].rearrange("p (s k) -> p s k", s=NSLIDES), Alu.mult,
            )
            nc.scalar.activation(vecS[0:1, V_DFT:V_DFT + 27], vecp[0:1, 20:47], Act.Copy)

            # preload the sqrt act-table before the collective so the post
            # stage (all sqrt_and_others funcs) pays no table load in the tail
            s2t = scp.tile([1, 1], F32, tag="s2t")
            nc.vector.tensor_tensor(s2t[:], sincos[0:1, 0:1], sincos[0:1, 0:1], Alu.mult)
            dum = scp.tile([1, 1], F32, tag="dum")
            dum_bi = nc.scalar.activation(dum[:], s2t[:], Act.Sqrt)
            bass._add_dep_helper(dum_bi.ins, sign_insts[-1], False,
                                 "order: sqrt table preload after collision signs")

            ccin = dr.tile([1, 128], F32)
            ccout = dr.tile([1, 128], F32, addr_space="Shared")
            nc.sync.dma_start(ccin[:], vecS[:])
            if collective:
                nc.gpsimd.collective_compute(
                    "AllReduce", Alu.add,
                    replica_groups=[list(range(N_CORES))],
                    ins=[ccin.opt()], outs=[ccout.opt()],
                )
            else:
                nc.sync.dma_start(ccout[:], ccin[:])
            vecR = cst.tile([1, 128], F32, tag="vecR")
            nc.sync.dma_start(vecR[:], ccout[:])

            # ---------------- descriptors + variance ----------------
            SC = cst.tile([1, 76], F32, tag="SC")
            # hist fractions (radial counts were pre-scaled by 1/N via the mask);
            # vec radial layout is s-major (s:3, j:21); output dims follow (s, j).
            rad63 = vecR[0:1, 0:63].rearrange("p (s j) -> p s j", s=NSLIDES)
            difv = SC[0:1, 0:60].rearrange("p (j s) -> p s j", j=20)
            nc.vector.tensor_tensor(difv, rad63[:, :, 1:21], rad63[:, :, 0:20], Alu.subtract)
            # power spectrum k=1..4: sqrt(cos^2 + sin^2)
            t24 = scp.tile([1, 24], F32, tag="t24")
            nc.vector.tensor_tensor(t24[:], vecR[0:1, V_DFT:V_DFT + 24],
                                    vecR[0:1, V_DFT:V_DFT + 24], Alu.mult)
            nc.vector.tensor_tensor(SC[0:1, 60:72], t24[0:1, 0:12], t24[0:1, 12:24], Alu.add)
            nc.scalar.activation(SC[0:1, 60:72], SC[0:1, 60:72], Act.Sqrt)
            # collision rates (pre-scaled at assembly)
            nc.vector.tensor_copy(SC[0:1, 72:75], vecR[0:1, V_COLL:V_COLL + 3])
            # variance over slides (ddof=1), mean over 26 components
            m25 = scp.tile([1, 25], F32, tag="m25")
            nc.vector.tensor_reduce(
                m25[:], SC[0:1, 0:75].rearrange("p (c s) -> p c s", c=25),
                mybir.AxisListType.X, Alu.add,
            )
            # dev = mean - x (sign-flipped; squared next)
            dev = scp.tile([1, 75], F32, tag="dev")
            nc.vector.scalar_tensor_tensor(
                dev[:].rearrange("p (c s) -> p c s", c=25),
                _bcast(m25[:], NSLIDES, 1), 1.0 / NSLIDES,
                SC[0:1, 0:75].rearrange("p (c s) -> p c s", c=25),
                Alu.mult, Alu.subtract,
            )
            nc.vector.tensor_tensor(dev[:], dev[:], dev[:], Alu.mult)
            tot = scp.tile([1, 1], F32, tag="tot")
            nc.vector.tensor_reduce(
                tot[:], dev[:].rearrange("p (c s) -> p c s", c=25),
                mybir.AxisListType.XY, Alu.add,
            )
            outS = scp.tile([1, 1], F32, tag="outS")
            nc.scalar.activation(outS[:], tot[:], Act.Copy, scale=1.0 / (2.0 * 26.0))
            nc.sync.dma_start(o_out[:], outS[:])

    nc.compile()
    return nc


_PROG_CACHE = {}


def _get_program(win):
    if win not in _PROG_CACHE:
        _PROG_CACHE[win] = build_program(win)
    return _PROG_CACHE[win]


def _host_prep(coords_list, win):
    whalf = (win - 128) // 2
    buf = ROWS_PER_CORE + win - 128
    bufp = buf // 128
    SENT_X = np.float32(1e6)

    O_PXY, O_RXY = 0, 384
    O_PW = O_RXY + 128
    O_XSH = O_PW + NSLIDES * 2 * bufp
    O_YSH = O_XSH + SH
    O_KVEC = O_YSH + SH
    O_K4 = O_KVEC + 20
    O_MASK = O_K4 + 4
    F1 = O_MASK + 64
    O_LHS = NSLIDES * buf
    F2 = O_LHS + NSLIDES * ROWS_PER_CORE

    sxy = []
    for c in coords_list:
        order = np.argsort(c[:, 0], kind="stable")
        sxy.append(np.ascontiguousarray(c[order]))

    base128 = np.zeros((128, F1), np.float32)
    for s in range(NSLIDES):
        base128[:, O_PXY + 128 * s:O_PXY + 128 * s + 64] = sxy[s][:, 0].reshape(128, 64)
        base128[:, O_PXY + 128 * s + 64:O_PXY + 128 * (s + 1)] = sxy[s][:, 1].reshape(128, 64)
    base128[:, O_KVEC:O_KVEC + 20] = (np.arange(1, 21, dtype=np.float32) / np.float32(20)) ** 2
    base128[:, O_K4:O_K4 + 4] = np.arange(1, 5, dtype=np.float32)

    in_maps = []
    for core in range(N_CORES):
        r0 = core * ROWS_PER_CORE
        m128 = base128.copy()
        m5 = np.zeros((5, F2), np.float32)
        for s in range(NSLIDES):
            xs, ys = sxy[s][:, 0], sxy[s][:, 1]
            sl = slice(O_LHS + s * ROWS_PER_CORE, O_LHS + (s + 1) * ROWS_PER_CORE)
            m5[0, sl] = xs[r0:r0 + ROWS_PER_CORE]
            m5[1, sl] = ys[r0:r0 + ROWS_PER_CORE]
            m5[2, sl] = m5[0, sl]
            m5[3, sl] = m5[1, sl]
            m5[4, sl] = -1.0
            # window buffer [r0-whalf, r0+1024+whalf) with sentinel padding
            xb = np.full(buf, SENT_X, np.float32)
            yb = np.zeros(buf, np.float32)
            g0 = r0 - whalf
            lo, hi = max(g0, 0), min(g0 + buf, N)
            xb[lo - g0:hi - g0] = xs[lo:hi]
            yb[lo - g0:hi - g0] = ys[lo:hi]
            m5[0, s * buf:(s + 1) * buf] = xb
            m5[1, s * buf:(s + 1) * buf] = yb
            m5[2, s * buf:(s + 1) * buf] = xb
            m5[3, s * buf:(s + 1) * buf] = yb
            m128[:, O_PW + 2 * bufp * s:O_PW + 2 * bufp * s + bufp] = xb.reshape(128, bufp)
            m128[:, O_PW + 2 * bufp * s + bufp:O_PW + 2 * bufp * (s + 1)] = yb.reshape(128, bufp)
            for b in range(NBLK):
                m128[:, O_XSH + s * NBLK + b] = xs[r0 + b * 128:r0 + b * 128 + 128]
                m128[:, O_YSH + s * NBLK + b] = ys[r0 + b * 128:r0 + b * 128 + 128]
        # radial: owned slide (cores 0-2), mask row 0
        m128[:, O_RXY:O_RXY + 64] = sxy[core % NSLIDES][:, 0].reshape(128, 64)
        m128[:, O_RXY + 64:O_RXY + 128] = sxy[core % NSLIDES][:, 1].reshape(128, 64)
        if core < NSLIDES:
            m128[0, O_MASK + core * 20:O_MASK + (core + 1) * 20] = np.float32(1.0) / np.float32(N)
        in_maps.append({"m128": m128, "m5": m5})
    return in_maps


def _pick_win(coords_list):
    # win > 2048 would need a deeper PSUM chunking scheme; these whalf values
    # cover any remotely Gaussian-like input (the shipped inputs pass at 64)
    for whalf in (64, 192, 448, 960):
        ok = True
        for c in coords_list:
            xs = np.sort(c[:, 0])
            if (xs[whalf:] - xs[:-whalf]).min() < 0.01:
                ok = False
                break
        if ok:
            return 128 + 2 * whalf
    raise ValueError("no valid rank window (pathological input)")


def kernel(coords0, coords1, coords2, slide_labels=None, **_):
    coords_list = [np.ascontiguousarray(np.asarray(c, dtype=np.float32))
                   for c in (coords0, coords1, coords2)]
    assert coords_list[0].shape == (N, 2)
    win = _pick_win(coords_list)
    nc = _get_program(win)
    in_maps = _host_prep(coords_list, win)
    res = run_bass_kernel_spmd(nc, in_maps, core_ids=list(range(N_CORES)))
    val = np.float32(res.results[0]["out"][0, 0])
    return np.asarray(val, dtype=np.float32).reshape(())



# revision 10
# speedup vs baseline: 1.0077x; 1.0077x over previous
"""Trainium2 Bass kernel for nn_CrossSlideConsistencyLoss.

Computes, for 3 slides of 8192 2-D points each:
  - radial histogram (20 bins) of centered radii
  - |FFT|[0:5] of the mean-centered angular histogram (72 bins)
  - collision rate: fraction of points whose nearest neighbor is < 0.01 away
then the mean over descriptor components of the across-slide variance (ddof=1).

Strategy (8 NeuronCores, SPMD):
  - Host sorts each slide's points by x (pure permutation; every descriptor
    piece is permutation invariant). Any pair closer than 0.01 is then within
    whalf ranks of each other (validated at runtime), so the NxN cdist
    collapses to a banded window per 128-row block. One K=6 matmul per block
    computes z' = 2 x_i x_j + 2 y_i y_j - sq_j - sq_i + th directly (the
    per-row bias rides in matmul rows 4/5 with host-precomputed sq), so
    d2 < th  <=>  z' > 0 uniformly: ACT counts via Sign+accum, DVE via
    is_gt+accum, both with zero bias. Rows are sharded over the 8 cores.
  - Angular DFT: only |FFT| bins 1..4 of the angular histogram are needed
    (bin 0 of the mean-centered histogram is ~0); they equal direct sums of
    cos/sin(2 pi k aidx / 72) over points - computed shard-local on every
    core and summed by the final AllReduce. The floor/range-reduce helper
    chain runs on the (otherwise idle) gpsimd engine.
  - Radial histogram: one core per slide bins it whole (mask-gated). Radii
    are rescaled to g = rf2 * 400 / (rmax2 + 1e-8) so all 19 bin thresholds
    become the constants k^2 (bin 20 is the known total = 1.0 after
    normalization); the compare+count runs as two fat fp16 2x-mode ops
    mid-stream on DVE.
  - One 512B AllReduce combines [radial counts | DFT sums | collision
    counts]; every core then computes the final variance; core 0's scalar
    output is returned.

Emission order is engine-schedule-aware: per-engine queues execute in order,
so instructions are emitted in the order each engine should run them
(earliest-ready first), with a few explicit cross-engine ordering deps.
"""
import numpy as np

import concourse.bass as bass
import concourse.bacc as bacc
import concourse.bass_isa as bass_isa
import concourse.mybir as mybir
import concourse.tile as tile
from concourse.bass_utils import run_bass_kernel_spmd

F32 = mybir.dt.float32
F32R = mybir.dt.float32r
F16 = mybir.dt.float16
Alu = mybir.AluOpType
Act = mybir.ActivationFunctionType

N = 8192
N_CORES = 8
NSLIDES = 3
ROWS_PER_CORE = N // N_CORES          # 1024
NBLK = ROWS_PER_CORE // 128           # 8 blocks per core per slide
SH = NSLIDES * NBLK                   # 24 shard columns
TH = 1e-4                             # d^2 threshold (0.01^2)
PI = float(np.pi)
R2C = 12582912.0                      # 1.5 * 2^23: rne magic constant
NRB = 19                              # compared radial cum-bins (bin 20 = 1)

# collision block assignment: DVE handles the earliest-produced blocks of
# slide 0 (ready before the center-dependent DVE chain can start) plus the
# tail blocks of slides 1-2; ACT takes the rest (15 blocks).
DVE_OF = {0: (0, 1, 2), 1: (5, 6, 7), 2: (5, 6, 7)}
N_EARLY = 3                           # (s0, b0..2) hoisted before the chain

# AllReduce vector layout ([1, 128] f32):
#   [0:63)   radial cumcounts, per slide: [C0=0, C1..C20] (21 cols x 3)
#   [63:87)  DFT sums: 12 cos (k-major, s-minor), then 12 sin
#   [87:90)  collision row counts per slide
V_DFT = 63
V_COLL = 87

# mega-input column layout for m128 [128, F1] (f32):
#   pxy(3*128) | rxy(128) | xsh(24) | ysh(24) | k4(4) | mask row0 (64)
#   | thr24(24) | kk2 fp16 pairs (19*64/2 = 608)
O_PXY, O_RXY = 0, 384
O_XSH = O_RXY + 128
O_YSH = O_XSH + SH
O_K4 = O_YSH + SH
O_MASK = O_K4 + 4
O_THR = O_MASK + 64
O_KK2 = O_THR + SH
F1 = O_KK2 + NRB * 32


def _bcast(ap, axis_len, at):
    """Insert a broadcast (stride-0) dim of length axis_len at free position
    `at` (0 = before the flattened free dim, 1 = after it)."""
    p, f = ap.shape[0], int(np.prod(ap.shape[1:]))
    if at == 0:
        return ap.rearrange("p (a b) -> p a b", a=1).to_broadcast([p, axis_len, f])
    return ap.rearrange("p (a b) -> p a b", b=1).to_broadcast([p, f, axis_len])


def build_program(win, collective=True):
    buf = ROWS_PER_CORE + win - 128   # rhs window buffer length per core/slide
    assert buf % 128 == 0

    # m6 [6, F2]: rhs buffers (x,y,x,y,sq,-1) | lhs (x,y,x,y,-1,sq-th)
    O_LHS = NSLIDES * buf
    F2 = O_LHS + NSLIDES * ROWS_PER_CORE

    nc = bacc.Bacc("TRN2", target_bir_lowering=False, debug=False, num_devices=N_CORES)
    i_m128 = nc.dram_tensor("m128", [128, F1], F32, kind="ExternalInput")
    i_m6 = nc.dram_tensor("m6", [6, F2], F32R, kind="ExternalInput")
    o_out = nc.dram_tensor("out", [1, 1], F32, kind="ExternalOutput")

    with tile.TileContext(nc) as tc:
        with (
            tc.tile_pool(name="cst", bufs=1) as cst,
            tc.tile_pool(name="scr", bufs=3) as scp,
            tc.tile_pool(name="psum",
                         bufs=max(1, 6 // max(1, win * 4 // 2048)),
                         space="PSUM") as pp,
            tc.tile_pool(name="psv", bufs=1, space="PSUM") as pv,
            tc.tile_pool(name="dram", bufs=1, space="DRAM") as dr,
        ):
            # ---------------- input loads (3 DMAs: pxy | m6 | rest) --------
            big128 = cst.tile([128, F1], F32, tag="big128")
            nc.sync.dma_start(big128[:, 0:O_RXY], i_m128[:, 0:O_RXY])
            big6 = cst.tile([6, F2], F32R, tag="big6")
            nc.sync.dma_start(big6[:], i_m6[:])
            nc.sync.dma_start(big128[:, O_RXY:F1], i_m128[:, O_RXY:F1])

            pxys = [big128[:, O_PXY + 128 * s:O_PXY + 128 * (s + 1)] for s in range(NSLIDES)]
            rxy = big128[:, O_RXY:O_RXY + 128]
            xsh = big128[:, O_XSH:O_XSH + SH]
            ysh = big128[:, O_YSH:O_YSH + SH]
            k4T = big128[:, O_K4:O_K4 + 4]
            maskR = big128[0:1, O_MASK:O_MASK + 64]
            thr24 = big128[:, O_THR:O_THR + SH]
            kk2 = big128[:, O_KK2:O_KK2 + NRB * 32].bitcast(F16)  # [128, 19*64]
            rhs = [big6[:, buf * s:buf * (s + 1)] for s in range(NSLIDES)]
            lhsT = big6[:, O_LHS:O_LHS + NSLIDES * ROWS_PER_CORE]

            ones128 = cst.tile([128, 1], F32, tag="ones128")
            nc.gpsimd.memset(ones128[:], 1.0)
            invN128 = cst.tile([128, 1], F32, tag="invN128")
            nc.gpsimd.memset(invN128[:], 1.0 / N)
            ones128h = cst.tile([128, 1], F16, tag="ones128h")
            nc.gpsimd.memset(ones128h[:], 1.0)
            vecS = cst.tile([1, 128], F32, tag="vecS")
            nc.gpsimd.memset(vecS[:], 0.0)
            nc.scalar.add_instruction(mybir.InstLoadActFuncSet(
                act_func_set_id=9, name=f"I-{nc.next_id()}", ins=[], outs=[]))

            acc = cst.tile([128, SH], F32, tag="acc")
            vecp = pv.tile([1, 64], F32, tag="vecp")
            sign_insts = []
            dve_cmp_insts = []

            def emit_block(s, b):
                col = s * NBLK + b
                zp = pp.tile([128, win], F32, tag="zp")
                for off in range(0, win, 512):  # moving free dim cap is 512
                    nc.tensor.matmul(
                        zp[:, off:off + min(512, win - off)],
                        lhsT[:, s * ROWS_PER_CORE + b * 128:
                             s * ROWS_PER_CORE + b * 128 + 128],
                        rhs[s][:, b * 128 + off:b * 128 + off + min(512, win - off)],
                    )
                if b in DVE_OF[s]:
                    sg = scp.tile([128, win], F32, tag="sgd")
                    dve_cmp_insts.append(nc.vector.tensor_scalar(
                        sg[:], zp[:], 0.0, None, Alu.is_gt,
                        Alu.add, accum_out=acc[:, col:col + 1],
                    ).ins)
                else:
                    sg = scp.tile([128, win], F32, tag="sg")
                    sign_insts.append(nc.scalar.activation(
                        sg[:], zp[:], Act.Sign, accum_out=acc[:, col:col + 1],
                    ).ins)

            # ---------------- centers (DVE reduces first) ------------------
            c8p = pv.tile([1, 8], F32, tag="c8p")
            csums = []
            for s in range(NSLIDES):
                csum = scp.tile([128, 2], F32, tag="csum")
                nc.vector.tensor_reduce(
                    csum[:], pxys[s].rearrange("p (t f) -> p t f", t=2),
                    mybir.AxisListType.X, Alu.add,
                )
                csums.append(csum)
            rsum = scp.tile([128, 2], F32, tag="csum")
            nc.vector.tensor_reduce(
                rsum[:], rxy.rearrange("p (t f) -> p t f", t=2),
                mybir.AxisListType.X, Alu.add,
            )

            # early collision blocks: DVE chews these while the center chain
            # is still in flight (their matmuls also head the PE queue)
            for b in range(N_EARLY):
                emit_block(0, b)

            for s in range(NSLIDES):
                nc.tensor.matmul(c8p[0:1, 2 * s:2 * s + 2], ones128[:], csums[s])
            nc.tensor.matmul(c8p[0:1, 6:8], ones128[:], rsum[:])

            crowX = cst.tile([1, SH], F32, tag="crowX")
            nc.scalar.activation(
                crowX[:].rearrange("p (s b) -> p s b", s=NSLIDES),
                c8p[0:1, 0:6].rearrange("p (s t) -> p s t", s=NSLIDES)[:, :, 0:1]
                .to_broadcast([1, NSLIDES, NBLK]),
                Act.Copy, scale=1.0 / N,
            )
            crowY = cst.tile([1, SH], F32, tag="crowY")
            nc.scalar.activation(
                crowY[:].rearrange("p (s b) -> p s b", s=NSLIDES),
                c8p[0:1, 0:6].rearrange("p (s t) -> p s t", s=NSLIDES)[:, :, 1:2]
                .to_broadcast([1, NSLIDES, NBLK]),
                Act.Copy, scale=1.0 / N,
            )
            c2t = cst.tile([1, 2], F32, tag="c2t")
            nc.scalar.activation(c2t[:], c8p[0:1, 6:8], Act.Copy, scale=1.0 / N)
            cX24 = cst.tile([128, SH], F32, tag="cX24")
            nc.gpsimd.partition_broadcast(cX24[:], crowX[:])
            cY24 = cst.tile([128, SH], F32, tag="cY24")
            nc.gpsimd.partition_broadcast(cY24[:], crowY[:])
            crB = cst.tile([128, 2], F32, tag="crB")
            nc.gpsimd.partition_broadcast(crB[:], c2t[:])

            # ---------------- shard center-relative chains (DVE) -----------
            dx = scp.tile([128, SH], F32, tag="dx")
            nc.vector.tensor_tensor(dx[:], xsh, cX24[:], Alu.subtract)
            dy = scp.tile([128, SH], F32, tag="dy")
            nc.vector.tensor_tensor(dy[:], ysh, cY24[:], Alu.subtract)
            r2 = scp.tile([128, SH], F32, tag="r2")
            nc.vector.tensor_tensor(r2[:], dx[:], dx[:], Alu.mult)
            yy = scp.tile([128, SH], F32, tag="yy")
            nc.vector.tensor_tensor(yy[:], dy[:], dy[:], Alu.mult)
            nc.vector.tensor_tensor(r2[:], r2[:], yy[:], Alu.add)
            I32 = mybir.dt.int32
            ih = scp.tile([128, SH], I32, tag="ih")
            nc.vector.tensor_scalar(ih[:], r2[:].bitcast(I32), 1, None, Alu.arith_shift_right)
            nc.vector.tensor_scalar(ih[:], ih[:], -1, 0x5F3759DF, Alu.mult, Alu.add)
            ny = scp.tile([128, SH], F32, tag="ny")
            nc.vector.tensor_copy(ny[:], ih[:].bitcast(F32))
            for _ in range(2):
                nt = scp.tile([128, SH], F32, tag="nt")
                nc.vector.tensor_tensor(nt[:], ny[:], ny[:], Alu.mult)
                nc.vector.scalar_tensor_tensor(nt[:], nt[:], -0.5, r2[:], Alu.mult, Alu.mult)
                nc.vector.scalar_tensor_tensor(ny[:], nt[:], 1.5, ny[:], Alu.add, Alu.mult)
            rr = scp.tile([128, SH], F32, tag="rr")
            nc.vector.tensor_tensor(rr[:], r2[:], ny[:], Alu.mult)

            # ---------------- radial + angle prep (gpsimd) -----------------
            dxf = scp.tile([128, 64], F32, tag="dxf")
            nc.gpsimd.tensor_scalar(dxf[:], rxy[:, 0:64], crB[:, 0:1], None, Alu.subtract)
            dyf = scp.tile([128, 64], F32, tag="dyf")
            nc.gpsimd.tensor_scalar(dyf[:], rxy[:, 64:128], crB[:, 1:2], None, Alu.subtract)
            rf2 = scp.tile([128, 64], F32, tag="rf2")
            nc.gpsimd.tensor_tensor(rf2[:], dxf[:], dxf[:], Alu.mult)
            yyf = scp.tile([128, 64], F32, tag="yyf")
            nc.gpsimd.tensor_tensor(yyf[:], dyf[:], dyf[:], Alu.mult)
            nc.gpsimd.tensor_tensor(rf2[:], rf2[:], yyf[:], Alu.add)
            neg = scp.tile([128, SH], F32, tag="neg")
            nc.gpsimd.tensor_scalar(neg[:], dx[:], 0.0, None, Alu.is_lt)
            c1 = scp.tile([128, SH], F32, tag="c1")
            nc.gpsimd.tensor_scalar(c1[:], neg[:], -2.0, 1.0, Alu.mult, Alu.add)
            sy36 = scp.tile([128, SH], F32, tag="sy36")
            nc.gpsimd.tensor_scalar(sy36[:], dy[:], 0.0, 72.0, Alu.is_ge, Alu.mult)
            nc.gpsimd.tensor_scalar(sy36[:], sy36[:], 36.0, None, Alu.subtract)
            pn36p = scp.tile([128, SH], F32, tag="pn36p")
            nc.gpsimd.tensor_tensor(pn36p[:], neg[:], sy36[:], Alu.mult)
            nc.gpsimd.tensor_scalar(pn36p[:], pn36p[:], 36.0, None, Alu.add)

            # ---------------- DVE mid chain --------------------------------
            rmx = scp.tile([128, 1], F32, tag="rmx")
            nc.vector.tensor_reduce(rmx[:], rf2[:], mybir.AxisListType.X, Alu.max)
            adx = scp.tile([128, SH], F32, tag="adx")
            adx_bi = nc.scalar.activation(adx[:], dx[:], Act.Abs)
            den = scp.tile([128, SH], F32, tag="den")
            nc.vector.scalar_tensor_tensor(den[:], rr[:], 1e-38, adx[:], Alu.add, Alu.add)
            rden = scp.tile([128, SH], F32, tag="rden")
            nc.vector.reciprocal(rden[:], den[:])
            qt = scp.tile([128, SH], F32, tag="qt")
            nc.vector.tensor_tensor(qt[:], dy[:], rden[:], Alu.mult)
            rmxB = scp.tile([128, 1], F32, tag="rmxB")
            nc.gpsimd.partition_all_reduce(rmxB[:], rmx[:], 128, bass_isa.ReduceOp.max)
            rm1 = scp.tile([128, 1], F32, tag="rm1")
            nc.vector.tensor_scalar(rm1[:], rmxB[:], 1e-8, None, Alu.add)
            rcp = scp.tile([128, 1], F32, tag="rcp")
            nc.vector.reciprocal(rcp[:], rm1[:])
            # g = rf2 * 400 / (rmax2 + 1e-8); bins are then g < k^2 (const)
            g16 = scp.tile([128, 64], F16, tag="g16")
            nc.vector.tensor_scalar(g16[:], rf2[:], rcp[:, 0:1], 400.0, Alu.mult, Alu.mult)

            at = scp.tile([128, SH], F32, tag="at")
            scv = scp.tile([128, 8 * SH], F32, tag="scv")
            mb = scp.tile([128, 8 * SH], F32, tag="mb")
            vb = scp.tile([128, 8 * SH], F32, tag="vb")

            def emit_dft_mid():
                # arctan (ACT) -> t1/ut (DVE) -> floor+range-reduce (Pool);
                # the Pool chain runs concurrently with the collision stream
                at_bi = nc.scalar.activation(at[:], qt[:], Act.Arctan)
                bass._add_dep_helper(at_bi.ins, sign_insts[-1], False,
                                     "order: arctan after 5 signs")
                t1 = scp.tile([128, SH], F32, tag="t1")
                nc.vector.tensor_tensor(t1[:], at[:], c1[:], Alu.mult)
                ut = scp.tile([128, SH], F32, tag="ut")
                nc.vector.scalar_tensor_tensor(ut[:], t1[:], 72.0 / PI, pn36p[:],
                                               Alu.mult, Alu.add)
                # floor + range-reduce: rv/vb/mb scalar stages on Pool (the
                # only tensor-tensor/STT forms the HW Pool engine accepts are
                # add/mult, so the compares and subtracts stay on DVE)
                rv = scp.tile([128, SH], F32, tag="rv")
                nc.gpsimd.tensor_scalar(rv[:], ut[:], R2C, R2C, Alu.add, Alu.subtract)
                cmp = scp.tile([128, SH], F32, tag="cmp")
                nc.vector.tensor_tensor(cmp[:], rv[:], ut[:], Alu.is_gt)
                nfl = scp.tile([128, SH], F32, tag="nfl")
                nc.vector.tensor_tensor(nfl[:], cmp[:], rv[:], Alu.subtract)
                ka = scp.tile([128, 4 * SH], F32, tag="ka")
                for k in range(1, 5):
                    nc.gpsimd.tensor_scalar(ka[:, (k - 1) * SH:k * SH], nfl[:],
                                            -71.0, float(k), Alu.max, Alu.mult)
                nc.gpsimd.tensor_scalar(vb[:, 0:4 * SH], ka[:], 1.0 / 72.0, None, Alu.mult)
                nc.gpsimd.tensor_scalar(vb[:, 4 * SH:8 * SH], ka[:], 1.0 / 72.0, 0.25,
                                        Alu.mult, Alu.add)
                nc.gpsimd.tensor_scalar(mb[:], vb[:], R2C, R2C, Alu.add, Alu.subtract)

            # ---------------- remaining collision blocks -------------------
            # ACT stream: signs with arctan/sin spliced in at readiness points
            nsign = 0
            sin_bi = None
            for s in range(NSLIDES):
                for b in range(NBLK):
                    if s == 0 and b < N_EARLY:
                        continue
                    emit_block(s, b)
                    if b not in DVE_OF[s]:
                        nsign += 1
                        if nsign == 5:
                            emit_dft_mid()
                        elif nsign == 12:
                            nc.vector.tensor_tensor(mb[:], vb[:], mb[:], Alu.subtract)
                            sin_bi = nc.scalar.activation(scv[:], mb[:], Act.Sin,
                                                          scale=2.0 * PI)
                            bass._add_dep_helper(sin_bi.ins, sign_insts[-1], False,
                                                 "order: sin after 12 signs")

            # ---------------- radial histogram (fp16, mid-stream) ----------
            ct = scp.tile([128, NRB * 64], F16, tag="ct")
            nc.vector.tensor_tensor(
                ct[:].rearrange("p (k f) -> p k f", k=NRB),
                _bcast(g16[:], NRB, 0),
                kk2.rearrange("p (k f) -> p k f", k=NRB), Alu.is_lt,
            )
            cr = scp.tile([128, NRB], F16, tag="cr")
            with nc.allow_low_precision(reason="fp16 radial counts <= 64 are exact"):
                nc.vector.tensor_reduce(
                    cr[:], ct[:].rearrange("p (k f) -> p k f", k=NRB),
                    mybir.AxisListType.X, Alu.add,
                )
            nc.tensor.matmul(vecp[0:1, 0:NRB], ones128h[:], cr[:])

            # ---------------- trig reduces + DFT matmul --------------------
            sincos = cst.tile([128, 24], F32, tag="sincos")
            nc.vector.tensor_reduce(
                sincos[:, 12:24].rearrange("p (k s) -> p k s", k=4),
                scv[:, 0:4 * SH].rearrange("p (k s b) -> p k s b", k=4, s=NSLIDES),
                mybir.AxisListType.X, Alu.add,
            )
            nc.vector.tensor_reduce(
                sincos[:, 0:12].rearrange("p (k s) -> p k s", k=4),
                scv[:, 4 * SH:8 * SH].rearrange("p (k s b) -> p k s b", k=4, s=NSLIDES),
                mybir.AxisListType.X, Alu.add,
            )
            nc.tensor.matmul(vecp[0:1, 20:44], ones128[:], sincos[:])

            # ---------------- collision indicators -------------------------
            # ACT cols: sumsign > 3-win <=> count >= 2; DVE cols: count > 1.5
            # (per-column thresholds ride in from the host via thr24)
            ind = scp.tile([128, SH], F32, tag="ind")
            nc.vector.tensor_tensor(ind[:], acc[:], thr24, Alu.is_gt)
            indR = scp.tile([128, NSLIDES], F32, tag="indR")
            nc.vector.tensor_reduce(
                indR[:], ind[:].rearrange("p (s b) -> p s b", s=NSLIDES),
                mybir.AxisListType.X, Alu.add,
            )
            nc.tensor.matmul(vecp[0:1, 44:47], invN128[:], indR[:])

            # ---------------- assemble + AllReduce -------------------------
            radview = vecS[0:1, 0:63].rearrange(
                "p (s k) -> p s k", s=NSLIDES, k=21
            )[:, :, 1:1 + NRB]
            nc.vector.tensor_tensor(
                radview,
                vecp[0:1, 0:NRB].rearrange("p (o k) -> p o k", o=1)
                .to_broadcast([1, NSLIDES, NRB]),
                maskR[0:1, 0:60].rearrange("p (s k) -> p s k", s=NSLIDES)[:, :, 0:NRB],
                Alu.mult,
            )
            # C20 = 1.0 for the owner core (counts < rmax2+1e-8 is all of them)
            c20v = vecS[0:1, 20:83].rearrange("p (s r) -> p s r", s=NSLIDES)[:, :, 0:1]
            m20v = maskR[0:1, 0:60].rearrange("p (s r) -> p s r", s=NSLIDES)[:, :, 0:1]
            nc.vector.tensor_scalar(c20v, m20v, float(N), None, Alu.mult)
            nc.vector.tensor_copy(vecS[0:1, V_DFT:V_DFT + 27], vecp[0:1, 20:47])

            # preload the sqrt act-table before the collective so the post
            # stage pays no table load in the tail
            s2t = scp.tile([1, 1], F32, tag="s2t")
            nc.vector.tensor_tensor(s2t[:], sincos[0:1, 0:1], sincos[0:1, 0:1], Alu.mult)
            dum = scp.tile([1, 1], F32, tag="dum")
            dum_bi = nc.scalar.activation(dum[:], s2t[:], Act.Sqrt)
            bass._add_dep_helper(dum_bi.ins, sign_insts[-1], False,
                                 "order: sqrt table preload after collision signs")
            bass._add_dep_helper(dum_bi.ins, sin_bi.ins, False,
                                 "order: sqrt table preload after sin")

            ccin = dr.tile([1, 128], F32)
            ccout = dr.tile([1, 128], F32, addr_space="Shared")
            nc.sync.dma_start(ccin[:], vecS[:])
            if collective:
                nc.gpsimd.collective_compute(
                    "AllReduce", Alu.add,
                    replica_groups=[list(range(N_CORES))],
                    ins=[ccin.opt()], outs=[ccout.opt()],
                )
            else:
                nc.sync.dma_start(ccout[:], ccin[:])
            vecR = cst.tile([1, 128], F32, tag="vecR")
            nc.sync.dma_start(vecR[:], ccout[:])

            # ---------------- descriptors + variance -----------------------
            SC = cst.tile([1, 76], F32, tag="SC")
            # hist fractions (radial counts were pre-scaled by 1/N via the mask);
            # vec radial layout is s-major (s:3, j:21); output dims follow (s, j).
            rad63 = vecR[0:1, 0:63].rearrange("p (s j) -> p s j", s=NSLIDES)
            difv = SC[0:1, 0:60].rearrange("p (j s) -> p s j", j=20)
            nc.vector.tensor_tensor(difv, rad63[:, :, 1:21], rad63[:, :, 0:20], Alu.subtract)
            # power spectrum k=1..4: sqrt(cos^2 + sin^2)
            t24 = scp.tile([1, 24], F32, tag="t24")
            nc.vector.tensor_tensor(t24[:], vecR[0:1, V_DFT:V_DFT + 24],
                                    vecR[0:1, V_DFT:V_DFT + 24], Alu.mult)
            nc.vector.tensor_tensor(SC[0:1, 60:72], t24[0:1, 0:12], t24[0:1, 12:24], Alu.add)
            nc.scalar.activation(SC[0:1, 60:72], SC[0:1, 60:72], Act.Sqrt)
            nc.vector.tensor_copy(SC[0:1, 72:75], vecR[0:1, V_COLL:V_COLL + 3])
            # variance over slides (ddof=1), mean over 26 components
            m25 = scp.tile([1, 25], F32, tag="m25")
            nc.vector.tensor_reduce(
                m25[:], SC[0:1, 0:75].rearrange("p (c s) -> p c s", c=25),
                mybir.AxisListType.X, Alu.add,
            )
            # dev = mean - x (sign-flipped; squared next)
            dev = scp.tile([1, 75], F32, tag="dev")
            nc.vector.scalar_tensor_tensor(
                dev[:].rearrange("p (c s) -> p c s", c=25),
                _bcast(m25[:], NSLIDES, 1), 1.0 / NSLIDES,
                SC[0:1, 0:75].rearrange("p (c s) -> p c s", c=25),
                Alu.mult, Alu.subtract,
            )
            nc.vector.tensor_tensor(dev[:], dev[:], dev[:], Alu.mult)
            tot = scp.tile([1, 1], F32, tag="tot")
            nc.vector.tensor_reduce(
                tot[:], dev[:].rearrange("p (c s) -> p c s", c=25),
                mybir.AxisListType.XY, Alu.add,
            )
            outS = scp.tile([1, 1], F32, tag="outS")
            nc.vector.tensor_scalar(outS[:], tot[:], 1.0 / (2.0 * 26.0), None, Alu.mult)
            nc.sync.dma_start(o_out[:], outS[:])

    nc.compile()
    return nc


_PROG_CACHE = {}


def _get_program(win):
    if win not in _PROG_CACHE:
        _PROG_CACHE[win] = build_program(win)
    return _PROG_CACHE[win]


def _host_prep(coords_list, win):
    whalf = (win - 128) // 2
    buf = ROWS_PER_CORE + win - 128
    SENT_X = np.float32(1e6)

    O_LHS = NSLIDES * buf
    F2 = O_LHS + NSLIDES * ROWS_PER_CORE

    sxy = []
    for c in coords_list:
        order = np.argsort(c[:, 0], kind="stable")
        sxy.append(np.ascontiguousarray(c[order]))

    base128 = np.zeros((128, F1), np.float32)
    for s in range(NSLIDES):
        base128[:, O_PXY + 128 * s:O_PXY + 128 * s + 64] = sxy[s][:, 0].reshape(128, 64)
        base128[:, O_PXY + 128 * s + 64:O_PXY + 128 * (s + 1)] = sxy[s][:, 1].reshape(128, 64)
    base128[:, O_K4:O_K4 + 4] = np.arange(1, 5, dtype=np.float32)
    # per-column collision indicator thresholds (ACT sign-sum vs DVE count)
    thr = np.empty(SH, np.float32)
    for s in range(NSLIDES):
        for b in range(NBLK):
            thr[s * NBLK + b] = np.float32(1.5 if b in DVE_OF[s] else 3.0 - win)
    base128[:, O_THR:O_THR + SH] = thr
    # radial bin thresholds k^2 (fp16 exact), bin-major, packed into f32 pairs
    kk2 = np.repeat((np.arange(1, NRB + 1, dtype=np.float16)) ** 2, 64)
    base128[:, O_KK2:O_KK2 + NRB * 32] = kk2.view(np.float32)[None, :]

    in_maps = []
    for core in range(N_CORES):
        r0 = core * ROWS_PER_CORE
        m128 = base128.copy()
        m6 = np.zeros((6, F2), np.float32)
        for s in range(NSLIDES):
            xs, ys = sxy[s][:, 0], sxy[s][:, 1]
            sl = slice(O_LHS + s * ROWS_PER_CORE, O_LHS + (s + 1) * ROWS_PER_CORE)
            xr = xs[r0:r0 + ROWS_PER_CORE]
            yr = ys[r0:r0 + ROWS_PER_CORE]
            m6[0, sl] = xr
            m6[1, sl] = yr
            m6[2, sl] = xr
            m6[3, sl] = yr
            m6[4, sl] = -1.0
            m6[5, sl] = (xr * xr + yr * yr) - np.float32(TH)
            # window buffer [r0-whalf, r0+1024+whalf) with sentinel padding
            xb = np.full(buf, SENT_X, np.float32)
            yb = np.zeros(buf, np.float32)
            g0 = r0 - whalf
            lo, hi = max(g0, 0), min(g0 + buf, N)
            xb[lo - g0:hi - g0] = xs[lo:hi]
            yb[lo - g0:hi - g0] = ys[lo:hi]
            m6[0, s * buf:(s + 1) * buf] = xb
            m6[1, s * buf:(s + 1) * buf] = yb
            m6[2, s * buf:(s + 1) * buf] = xb
            m6[3, s * buf:(s + 1) * buf] = yb
            m6[4, s * buf:(s + 1) * buf] = xb * xb + yb * yb
            m6[5, s * buf:(s + 1) * buf] = -1.0
            for b in range(NBLK):
                m128[:, O_XSH + s * NBLK + b] = xs[r0 + b * 128:r0 + b * 128 + 128]
                m128[:, O_YSH + s * NBLK + b] = ys[r0 + b * 128:r0 + b * 128 + 128]
        # radial: owned slide (cores 0-2), mask row 0
        m128[:, O_RXY:O_RXY + 64] = sxy[core % NSLIDES][:, 0].reshape(128, 64)
        m128[:, O_RXY + 64:O_RXY + 128] = sxy[core % NSLIDES][:, 1].reshape(128, 64)
        if core < NSLIDES:
            m128[0, O_MASK + core * 20:O_MASK + (core + 1) * 20] = np.float32(1.0) / np.float32(N)
        in_maps.append({"m128": m128, "m6": m6})
    return in_maps


def _pick_win(coords_list):
    # win > 2048 would need a deeper PSUM chunking scheme; these whalf values
    # cover any remotely Gaussian-like input (the shipped inputs pass at 64)
    for whalf in (64, 192, 448, 960):
        ok = True
        for c in coords_list:
            xs = np.sort(c[:, 0])
            if (xs[whalf:] - xs[:-whalf]).min() < 0.01:
                ok = False
                break
        if ok:
            return 128 + 2 * whalf
    raise ValueError("no valid rank window (pathological input)")


def kernel(coords0, coords1, coords2, slide_labels=None, **_):
    coords_list = [np.ascontiguousarray(np.asarray(c, dtype=np.float32))
                   for c in (coords0, coords1, coords2)]
    assert coords_list[0].shape == (N, 2)
    win = _pick_win(coords_list)
    nc = _get_program(win)
    in_maps = _host_prep(coords_list, win)
    res = run_bass_kernel_spmd(nc, in_maps, core_ids=list(range(N_CORES)))
    val = np.float32(res.results[0]["out"][0, 0])
    return np.asarray(val, dtype=np.float32).reshape(())


# revision 12
# speedup vs baseline: 1.1553x; 1.1465x over previous
"""Trainium2 Bass kernel for nn_CrossSlideConsistencyLoss.

Computes, for 3 slides of 8192 2-D points each:
  - radial histogram (20 bins) of centered radii
  - |FFT|[0:5] of the mean-centered angular histogram (72 bins)
  - collision rate: fraction of points whose nearest neighbor is < 0.01 away
then the mean over descriptor components of the across-slide variance (ddof=1).

Strategy (8 NeuronCores, SPMD):
  - Host sorts each slide's points by x (pure permutation; every descriptor
    piece is permutation invariant). Any pair closer than 0.01 is then within
    whalf ranks of each other (validated at runtime), so the NxN cdist
    collapses to a banded window per 128-row block. One K=6 matmul per block
    computes z' = 2 x_i x_j + 2 y_i y_j - sq_j - sq_i + th directly (the
    per-row bias rides in matmul rows 4/5 with host-precomputed sq), so
    d2 < th  <=>  z' > 0 uniformly: ACT counts via Sign+accum, DVE via
    is_gt+accum, both with zero bias. Rows are sharded over the 8 cores.
  - Angular DFT: only |FFT| bins 1..4 of the angular histogram are needed
    (bin 0 of the mean-centered histogram is ~0); they equal direct sums of
    cos/sin(2 pi k aidx / 72) over points - computed shard-local on every
    core and summed by the final AllReduce. The floor/range-reduce helper
    chain runs on the (otherwise idle) gpsimd engine.
  - Radial histogram: one core per slide bins it whole (mask-gated). Radii
    are rescaled to g = rf2 * 400 / (rmax2 + 1e-8) so all 19 bin thresholds
    become the constants k^2 (bin 20 is the known total = 1.0 after
    normalization); the compare+count runs as two fat fp16 2x-mode ops
    mid-stream on DVE.
  - One 512B AllReduce combines [radial counts | DFT sums | collision
    counts]; every core then computes the final variance; core 0's scalar
    output is returned.

Emission order is engine-schedule-aware: per-engine queues execute in order,
so instructions are emitted in the order each engine should run them
(earliest-ready first), with a few explicit cross-engine ordering deps.
"""
import numpy as np

import concourse.bass as bass
import concourse.bacc as bacc
import concourse.bass_isa as bass_isa
import concourse.mybir as mybir
import concourse.tile as tile
from concourse.bass_utils import run_bass_kernel_spmd

F32 = mybir.dt.float32
F32R = mybir.dt.float32r
F16 = mybir.dt.float16
Alu = mybir.AluOpType
Act = mybir.ActivationFunctionType

N = 8192
N_CORES = 8
NSLIDES = 3
ROWS_PER_CORE = N // N_CORES          # 1024
NBLK = ROWS_PER_CORE // 128           # 8 blocks per core per slide
SH = NSLIDES * NBLK                   # 24 shard columns
TH = 1e-4                             # d^2 threshold (0.01^2)
PI = float(np.pi)
R2C = 12582912.0                      # 1.5 * 2^23: rne magic constant
NRB = 19                              # compared radial cum-bins (bin 20 = 1)

# collision block assignment: DVE handles the earliest-produced blocks of
# slide 0 (ready before the center-dependent DVE chain can start) plus the
# tail blocks of slides 1-2; ACT takes the rest (15 blocks).
DVE_OF = {0: (0, 1, 2), 1: (5, 6, 7), 2: (5, 6, 7)}
N_EARLY = 3                           # (s0, b0..2) hoisted before the chain

# AllReduce vector layout ([1, 128] f32):
#   [0:63)   radial cumcounts, per slide: [C0=0, C1..C20] (21 cols x 3)
#   [63:87)  DFT sums: 12 cos (k-major, s-minor), then 12 sin
#   [87:90)  collision row counts per slide
V_DFT = 63
V_COLL = 87

# mega-input column layout for m128 [128, F1] (f32); slide coords arrive
# pre-centered from the host (distances in m6 use the raw coords)
#   xsh(24) | ysh(24) | thr24(24) | rxy(128) | mask row0 (64)
#   | kk2 fp16 pairs (19*64/2 = 608)
O_XSH = 0
O_YSH = O_XSH + SH
O_THR = O_YSH + SH
O_RXY = O_THR + SH
O_MASK = O_RXY + 128
O_KK2 = O_MASK + 64
F1 = O_KK2 + NRB * 32


def _bcast(ap, axis_len, at):
    """Insert a broadcast (stride-0) dim of length axis_len at free position
    `at` (0 = before the flattened free dim, 1 = after it)."""
    p, f = ap.shape[0], int(np.prod(ap.shape[1:]))
    if at == 0:
        return ap.rearrange("p (a b) -> p a b", a=1).to_broadcast([p, axis_len, f])
    return ap.rearrange("p (a b) -> p a b", b=1).to_broadcast([p, f, axis_len])


def build_program(win, collective=True):
    buf = ROWS_PER_CORE + win - 128   # rhs window buffer length per core/slide
    assert buf % 128 == 0

    # m6 [6, F2]: rhs buffers (x,y,x,y,sq,-1) | lhs (x,y,x,y,-1,sq-th)
    O_LHS = NSLIDES * buf
    F2 = O_LHS + NSLIDES * ROWS_PER_CORE

    nc = bacc.Bacc("TRN2", target_bir_lowering=False, debug=False, num_devices=N_CORES)
    i_m128 = nc.dram_tensor("m128", [128, F1], F32, kind="ExternalInput")
    i_m6 = nc.dram_tensor("m6", [6, F2], F32R, kind="ExternalInput")
    o_out = nc.dram_tensor("out", [1, 1], F32, kind="ExternalOutput")

    with tile.TileContext(nc) as tc:
        with (
            tc.tile_pool(name="cst", bufs=1) as cst,
            tc.tile_pool(name="scr", bufs=3) as scp,
            tc.tile_pool(name="psum",
                         bufs=max(1, 6 // max(1, win * 4 // 2048)),
                         space="PSUM") as pp,
            tc.tile_pool(name="psv", bufs=1, space="PSUM") as pv,
            tc.tile_pool(name="dram", bufs=1, space="DRAM") as dr,
        ):
            # ------------- input loads (3 DMAs: shard | m6 | rest) ---------
            big128 = cst.tile([128, F1], F32, tag="big128")
            nc.sync.dma_start(big128[:, 0:O_RXY], i_m128[:, 0:O_RXY])
            big6 = cst.tile([6, F2], F32R, tag="big6")
            nc.sync.dma_start(big6[:], i_m6[:])
            nc.sync.dma_start(big128[:, O_RXY:F1], i_m128[:, O_RXY:F1])


            rxy = big128[:, O_RXY:O_RXY + 128]
            xsh = big128[:, O_XSH:O_XSH + SH]
            ysh = big128[:, O_YSH:O_YSH + SH]
            maskR = big128[0:1, O_MASK:O_MASK + 64]
            thr24 = big128[:, O_THR:O_THR + SH]
            kk2 = big128[:, O_KK2:O_KK2 + NRB * 32].bitcast(F16)  # [128, 19*64]
            rhs = [big6[:, buf * s:buf * (s + 1)] for s in range(NSLIDES)]
            lhsT = big6[:, O_LHS:O_LHS + NSLIDES * ROWS_PER_CORE]

            ones128 = cst.tile([128, 1], F32, tag="ones128")
            nc.gpsimd.memset(ones128[:], 1.0)
            invN128 = cst.tile([128, 1], F32, tag="invN128")
            nc.gpsimd.memset(invN128[:], 1.0 / N)
            ones128h = cst.tile([128, 1], F16, tag="ones128h")
            nc.gpsimd.memset(ones128h[:], 1.0)
            vecS = cst.tile([1, 128], F32, tag="vecS")
            nc.gpsimd.memset(vecS[:], 0.0)
            nc.scalar.add_instruction(mybir.InstLoadActFuncSet(
                act_func_set_id=9, name=f"I-{nc.next_id()}", ins=[], outs=[]))

            acc = cst.tile([128, SH], F32, tag="acc")
            vecp = pv.tile([1, 64], F32, tag="vecp")
            sign_insts = []
            dve_cmp_insts = []

            def emit_block(s, b):
                col = s * NBLK + b
                zp = pp.tile([128, win], F32, tag="zp")
                for off in range(0, win, 512):  # moving free dim cap is 512
                    nc.tensor.matmul(
                        zp[:, off:off + min(512, win - off)],
                        lhsT[:, s * ROWS_PER_CORE + b * 128:
                             s * ROWS_PER_CORE + b * 128 + 128],
                        rhs[s][:, b * 128 + off:b * 128 + off + min(512, win - off)],
                    )
                if b in DVE_OF[s]:
                    sg = scp.tile([128, win], F32, tag="sgd")
                    dve_cmp_insts.append(nc.vector.tensor_scalar(
                        sg[:], zp[:], 0.0, None, Alu.is_gt,
                        Alu.add, accum_out=acc[:, col:col + 1],
                    ).ins)
                else:
                    sg = scp.tile([128, win], F32, tag="sg")
                    sign_insts.append(nc.scalar.activation(
                        sg[:], zp[:], Act.Sign, accum_out=acc[:, col:col + 1],
                    ).ins)

            # early collision blocks: DVE chews these while the input DMAs for
            # the shard chain are still in flight
            for b in range(N_EARLY):
                emit_block(0, b)

            # ---------------- shard chains (coords pre-centered) -----------
            dx = xsh
            dy = ysh
            r2 = scp.tile([128, SH], F32, tag="r2")
            nc.vector.tensor_tensor(r2[:], dx[:], dx[:], Alu.mult)
            yy = scp.tile([128, SH], F32, tag="yy")
            nc.vector.tensor_tensor(yy[:], dy[:], dy[:], Alu.mult)
            nc.vector.tensor_tensor(r2[:], r2[:], yy[:], Alu.add)
            I32 = mybir.dt.int32
            ih = scp.tile([128, SH], I32, tag="ih")
            nc.vector.tensor_scalar(ih[:], r2[:].bitcast(I32), 1, None, Alu.arith_shift_right)
            nc.vector.tensor_scalar(ih[:], ih[:], -1, 0x5F3759DF, Alu.mult, Alu.add)
            ny = scp.tile([128, SH], F32, tag="ny")
            nc.vector.tensor_copy(ny[:], ih[:].bitcast(F32))
            for _ in range(2):
                nt = scp.tile([128, SH], F32, tag="nt")
                nc.vector.tensor_tensor(nt[:], ny[:], ny[:], Alu.mult)
                nc.vector.scalar_tensor_tensor(nt[:], nt[:], -0.5, r2[:], Alu.mult, Alu.mult)
                nc.vector.scalar_tensor_tensor(ny[:], nt[:], 1.5, ny[:], Alu.add, Alu.mult)
            rr = scp.tile([128, SH], F32, tag="rr")
            nc.vector.tensor_tensor(rr[:], r2[:], ny[:], Alu.mult)

            # ---------------- radial + angle prep (gpsimd) -----------------
            dxf = rxy[:, 0:64]
            dyf = rxy[:, 64:128]
            rf2 = scp.tile([128, 64], F32, tag="rf2")
            nc.gpsimd.tensor_tensor(rf2[:], dxf, dxf, Alu.mult)
            yyf = scp.tile([128, 64], F32, tag="yyf")
            nc.gpsimd.tensor_tensor(yyf[:], dyf, dyf, Alu.mult)
            nc.gpsimd.tensor_tensor(rf2[:], rf2[:], yyf[:], Alu.add)
            neg = scp.tile([128, SH], F32, tag="neg")
            nc.gpsimd.tensor_scalar(neg[:], dx, 0.0, None, Alu.is_lt)
            c1 = scp.tile([128, SH], F32, tag="c1")
            nc.gpsimd.tensor_scalar(c1[:], neg[:], -2.0, 1.0, Alu.mult, Alu.add)
            sy36 = scp.tile([128, SH], F32, tag="sy36")
            nc.gpsimd.tensor_scalar(sy36[:], dy, 0.0, 72.0, Alu.is_ge, Alu.mult)
            nc.gpsimd.tensor_scalar(sy36[:], sy36[:], 36.0, None, Alu.subtract)
            pn36p = scp.tile([128, SH], F32, tag="pn36p")
            nc.gpsimd.tensor_tensor(pn36p[:], neg[:], sy36[:], Alu.mult)
            nc.gpsimd.tensor_scalar(pn36p[:], pn36p[:], 36.0, None, Alu.add)

            # ---------------- DVE mid chain --------------------------------
            rmx = scp.tile([128, 1], F32, tag="rmx")
            nc.vector.tensor_reduce(rmx[:], rf2[:], mybir.AxisListType.X, Alu.max)
            adx = scp.tile([128, SH], F32, tag="adx")
            adx_bi = nc.scalar.activation(adx[:], dx, Act.Abs)
            den = scp.tile([128, SH], F32, tag="den")
            nc.vector.scalar_tensor_tensor(den[:], rr[:], 1e-38, adx[:], Alu.add, Alu.add)
            rden = scp.tile([128, SH], F32, tag="rden")
            nc.vector.reciprocal(rden[:], den[:])
            qt = scp.tile([128, SH], F32, tag="qt")
            nc.vector.tensor_tensor(qt[:], dy, rden[:], Alu.mult)
            rmxB = scp.tile([128, 1], F32, tag="rmxB")
            nc.gpsimd.partition_all_reduce(rmxB[:], rmx[:], 128, bass_isa.ReduceOp.max)
            rm1 = scp.tile([128, 1], F32, tag="rm1")
            nc.vector.tensor_scalar(rm1[:], rmxB[:], 1e-8, None, Alu.add)
            rcp = scp.tile([128, 1], F32, tag="rcp")
            nc.vector.reciprocal(rcp[:], rm1[:])
            # g = rf2 * 400 / (rmax2 + 1e-8); bins are then g < k^2 (const)
            g16 = scp.tile([128, 64], F16, tag="g16")
            nc.vector.tensor_scalar(g16[:], rf2[:], rcp[:, 0:1], 400.0, Alu.mult, Alu.mult)

            at = scp.tile([128, SH], F32, tag="at")
            scv = scp.tile([128, 8 * SH], F32, tag="scv")
            mb = scp.tile([128, 8 * SH], F32, tag="mb")
            vb = scp.tile([128, 8 * SH], F32, tag="vb")

            def emit_dft_mid():
                # arctan (ACT) -> t1/ut (DVE) -> floor+range-reduce (Pool);
                # the Pool chain runs concurrently with the collision stream
                at_bi = nc.scalar.activation(at[:], qt[:], Act.Arctan)
                bass._add_dep_helper(at_bi.ins, sign_insts[-1], False,
                                     "order: arctan after 5 signs")
                t1 = scp.tile([128, SH], F32, tag="t1")
                nc.vector.tensor_tensor(t1[:], at[:], c1[:], Alu.mult)
                ut = scp.tile([128, SH], F32, tag="ut")
                nc.vector.scalar_tensor_tensor(ut[:], t1[:], 72.0 / PI, pn36p[:],
                                               Alu.mult, Alu.add)
                # floor + range-reduce: rv/vb/mb scalar stages on Pool (the
                # only tensor-tensor/STT forms the HW Pool engine accepts are
                # add/mult, so the compares and subtracts stay on DVE)
                rv = scp.tile([128, SH], F32, tag="rv")
                nc.gpsimd.tensor_scalar(rv[:], ut[:], R2C, R2C, Alu.add, Alu.subtract)
                cmp = scp.tile([128, SH], F32, tag="cmp")
                nc.vector.tensor_tensor(cmp[:], rv[:], ut[:], Alu.is_gt)
                nfl = scp.tile([128, SH], F32, tag="nfl")
                nc.vector.tensor_tensor(nfl[:], cmp[:], rv[:], Alu.subtract)
                ka = scp.tile([128, 4 * SH], F32, tag="ka")
                for k in range(1, 5):
                    nc.gpsimd.tensor_scalar(ka[:, (k - 1) * SH:k * SH], nfl[:],
                                            -71.0, float(k), Alu.max, Alu.mult)
                nc.gpsimd.tensor_scalar(vb[:, 0:4 * SH], ka[:], 1.0 / 72.0, None, Alu.mult)
                nc.gpsimd.tensor_scalar(vb[:, 4 * SH:8 * SH], ka[:], 1.0 / 72.0, 0.25,
                                        Alu.mult, Alu.add)
                nc.gpsimd.tensor_scalar(mb[:], vb[:], R2C, R2C, Alu.add, Alu.subtract)

            # ---------------- remaining collision blocks -------------------
            # ACT stream: signs with arctan/sin spliced in at readiness points
            nsign = 0
            sin_bi = None
            for s in range(NSLIDES):
                for b in range(NBLK):
                    if s == 0 and b < N_EARLY:
                        continue
                    emit_block(s, b)
                    if b not in DVE_OF[s]:
                        nsign += 1
                        if nsign == 5:
                            emit_dft_mid()
                        elif nsign == 12:
                            nc.vector.tensor_tensor(mb[:], vb[:], mb[:], Alu.subtract)
                            sin_bi = nc.scalar.activation(scv[:], mb[:], Act.Sin,
                                                          scale=2.0 * PI)
                            bass._add_dep_helper(sin_bi.ins, sign_insts[-1], False,
                                                 "order: sin after 12 signs")

            # ---------------- radial histogram (fp16, mid-stream) ----------
            ct = scp.tile([128, NRB * 64], F16, tag="ct")
            nc.vector.tensor_tensor(
                ct[:].rearrange("p (k f) -> p k f", k=NRB),
                _bcast(g16[:], NRB, 0),
                kk2.rearrange("p (k f) -> p k f", k=NRB), Alu.is_lt,
            )
            cr = scp.tile([128, NRB], F16, tag="cr")
            with nc.allow_low_precision(reason="fp16 radial counts <= 64 are exact"):
                nc.vector.tensor_reduce(
                    cr[:], ct[:].rearrange("p (k f) -> p k f", k=NRB),
                    mybir.AxisListType.X, Alu.add,
                )
            nc.tensor.matmul(vecp[0:1, 0:NRB], ones128h[:], cr[:])

            # ---------------- trig reduces + DFT matmul --------------------
            sincos = cst.tile([128, 24], F32, tag="sincos")
            nc.vector.tensor_reduce(
                sincos[:, 12:24].rearrange("p (k s) -> p k s", k=4),
                scv[:, 0:4 * SH].rearrange("p (k s b) -> p k s b", k=4, s=NSLIDES),
                mybir.AxisListType.X, Alu.add,
            )
            nc.vector.tensor_reduce(
                sincos[:, 0:12].rearrange("p (k s) -> p k s", k=4),
                scv[:, 4 * SH:8 * SH].rearrange("p (k s b) -> p k s b", k=4, s=NSLIDES),
                mybir.AxisListType.X, Alu.add,
            )
            nc.tensor.matmul(vecp[0:1, 20:44], ones128[:], sincos[:])

            # ---------------- collision indicators -------------------------
            # ACT cols: sumsign > 3-win <=> count >= 2; DVE cols: count > 1.5
            # (per-column thresholds ride in from the host via thr24)
            ind = scp.tile([128, SH], F32, tag="ind")
            nc.vector.tensor_tensor(ind[:], acc[:], thr24, Alu.is_gt)
            indR = scp.tile([128, NSLIDES], F32, tag="indR")
            nc.vector.tensor_reduce(
                indR[:], ind[:].rearrange("p (s b) -> p s b", s=NSLIDES),
                mybir.AxisListType.X, Alu.add,
            )
            nc.tensor.matmul(vecp[0:1, 44:47], invN128[:], indR[:])

            # ---------------- assemble + AllReduce -------------------------
            radview = vecS[0:1, 0:63].rearrange(
                "p (s k) -> p s k", s=NSLIDES, k=21
            )[:, :, 1:1 + NRB]
            nc.vector.tensor_tensor(
                radview,
                vecp[0:1, 0:NRB].rearrange("p (o k) -> p o k", o=1)
                .to_broadcast([1, NSLIDES, NRB]),
                maskR[0:1, 0:60].rearrange("p (s k) -> p s k", s=NSLIDES)[:, :, 0:NRB],
                Alu.mult,
            )
            # C20 = 1.0 for the owner core (counts < rmax2+1e-8 is all of them)
            c20v = vecS[0:1, 20:83].rearrange("p (s r) -> p s r", s=NSLIDES)[:, :, 0:1]
            m20v = maskR[0:1, 0:60].rearrange("p (s r) -> p s r", s=NSLIDES)[:, :, 0:1]
            nc.vector.tensor_scalar(c20v, m20v, float(N), None, Alu.mult)
            nc.vector.tensor_copy(vecS[0:1, V_DFT:V_DFT + 27], vecp[0:1, 20:47])

            # preload the sqrt act-table before the collective so the post
            # stage pays no table load in the tail
            s2t = scp.tile([1, 1], F32, tag="s2t")
            nc.vector.tensor_tensor(s2t[:], sincos[0:1, 0:1], sincos[0:1, 0:1], Alu.mult)
            dum = scp.tile([1, 1], F32, tag="dum")
            dum_bi = nc.scalar.activation(dum[:], s2t[:], Act.Sqrt)
            bass._add_dep_helper(dum_bi.ins, sign_insts[-1], False,
                                 "order: sqrt table preload after collision signs")
            bass._add_dep_helper(dum_bi.ins, sin_bi.ins, False,
                                 "order: sqrt table preload after sin")

            ccin = dr.tile([1, 128], F32)
            ccout = dr.tile([1, 128], F32, addr_space="Shared")
            nc.sync.dma_start(ccin[:], vecS[:])
            if collective:
                nc.gpsimd.collective_compute(
                    "AllReduce", Alu.add,
                    replica_groups=[list(range(N_CORES))],
                    ins=[ccin.opt()], outs=[ccout.opt()],
                )
            else:
                nc.sync.dma_start(ccout[:], ccin[:])
            vecR = cst.tile([1, 128], F32, tag="vecR")
            nc.sync.dma_start(vecR[:], ccout[:])

            # ---------------- descriptors + variance -----------------------
            SC = cst.tile([1, 76], F32, tag="SC")
            # hist fractions (radial counts were pre-scaled by 1/N via the mask);
            # vec radial layout is s-major (s:3, j:21); output dims follow (s, j).
            rad63 = vecR[0:1, 0:63].rearrange("p (s j) -> p s j", s=NSLIDES)
            difv = SC[0:1, 0:60].rearrange("p (j s) -> p s j", j=20)
            nc.vector.tensor_tensor(difv, rad63[:, :, 1:21], rad63[:, :, 0:20], Alu.subtract)
            # power spectrum k=1..4: sqrt(cos^2 + sin^2)
            t24 = scp.tile([1, 24], F32, tag="t24")
            nc.vector.tensor_tensor(t24[:], vecR[0:1, V_DFT:V_DFT + 24],
                                    vecR[0:1, V_DFT:V_DFT + 24], Alu.mult)
            nc.vector.tensor_tensor(SC[0:1, 60:72], t24[0:1, 0:12], t24[0:1, 12:24], Alu.add)
            nc.scalar.activation(SC[0:1, 60:72], SC[0:1, 60:72], Act.Sqrt)
            nc.vector.tensor_copy(SC[0:1, 72:75], vecR[0:1, V_COLL:V_COLL + 3])
            # variance over slides (ddof=1), mean over 26 components
            m25 = scp.tile([1, 25], F32, tag="m25")
            nc.vector.tensor_reduce(
                m25[:], SC[0:1, 0:75].rearrange("p (c s) -> p c s", c=25),
                mybir.AxisListType.X, Alu.add,
            )
            # dev = mean - x (sign-flipped; squared next)
            dev = scp.tile([1, 75], F32, tag="dev")
            nc.vector.scalar_tensor_tensor(
                dev[:].rearrange("p (c s) -> p c s", c=25),
                _bcast(m25[:], NSLIDES, 1), 1.0 / NSLIDES,
                SC[0:1, 0:75].rearrange("p (c s) -> p c s", c=25),
                Alu.mult, Alu.subtract,
            )
            nc.vector.tensor_tensor(dev[:], dev[:], dev[:], Alu.mult)
            tot = scp.tile([1, 1], F32, tag="tot")
            nc.vector.tensor_reduce(
                tot[:], dev[:].rearrange("p (c s) -> p c s", c=25),
                mybir.AxisListType.XY, Alu.add,
            )
            outS = scp.tile([1, 1], F32, tag="outS")
            nc.vector.tensor_scalar(outS[:], tot[:], 1.0 / (2.0 * 26.0), None, Alu.mult)
            nc.sync.dma_start(o_out[:], outS[:])

    nc.compile()
    return nc


_PROG_CACHE = {}


def _get_program(win):
    if win not in _PROG_CACHE:
        _PROG_CACHE[win] = build_program(win)
    return _PROG_CACHE[win]


def _host_prep(coords_list, win):
    whalf = (win - 128) // 2
    buf = ROWS_PER_CORE + win - 128
    SENT_X = np.float32(1e6)

    O_LHS = NSLIDES * buf
    F2 = O_LHS + NSLIDES * ROWS_PER_CORE

    sxy = []
    cxy = []
    for c in coords_list:
        order = np.argsort(c[:, 0], kind="stable")
        s = np.ascontiguousarray(c[order])
        sxy.append(s)
        # pre-centered copy for the histogram/DFT paths (f32 center, matching
        # the reference's coords.mean(axis=0) up to summation order)
        center = c.astype(np.float64).mean(axis=0).astype(np.float32)
        cxy.append((s - center).astype(np.float32))

    base128 = np.zeros((128, F1), np.float32)
    # per-column collision indicator thresholds (ACT sign-sum vs DVE count)
    thr = np.empty(SH, np.float32)
    for s in range(NSLIDES):
        for b in range(NBLK):
            thr[s * NBLK + b] = np.float32(1.5 if b in DVE_OF[s] else 3.0 - win)
    base128[:, O_THR:O_THR + SH] = thr
    # radial bin thresholds k^2 (fp16 exact), bin-major, packed into f32 pairs
    kk2 = np.repeat((np.arange(1, NRB + 1, dtype=np.float16)) ** 2, 64)
    base128[:, O_KK2:O_KK2 + NRB * 32] = kk2.view(np.float32)[None, :]

    in_maps = []
    for core in range(N_CORES):
        r0 = core * ROWS_PER_CORE
        m128 = base128.copy()
        m6 = np.zeros((6, F2), np.float32)
        for s in range(NSLIDES):
            xs, ys = sxy[s][:, 0], sxy[s][:, 1]
            sl = slice(O_LHS + s * ROWS_PER_CORE, O_LHS + (s + 1) * ROWS_PER_CORE)
            xr = xs[r0:r0 + ROWS_PER_CORE]
            yr = ys[r0:r0 + ROWS_PER_CORE]
            m6[0, sl] = xr
            m6[1, sl] = yr
            m6[2, sl] = xr
            m6[3, sl] = yr
            m6[4, sl] = -1.0
            m6[5, sl] = (xr * xr + yr * yr) - np.float32(TH)
            # window buffer [r0-whalf, r0+1024+whalf) with sentinel padding
            xb = np.full(buf, SENT_X, np.float32)
            yb = np.zeros(buf, np.float32)
            g0 = r0 - whalf
            lo, hi = max(g0, 0), min(g0 + buf, N)
            xb[lo - g0:hi - g0] = xs[lo:hi]
            yb[lo - g0:hi - g0] = ys[lo:hi]
            m6[0, s * buf:(s + 1) * buf] = xb
            m6[1, s * buf:(s + 1) * buf] = yb
            m6[2, s * buf:(s + 1) * buf] = xb
            m6[3, s * buf:(s + 1) * buf] = yb
            m6[4, s * buf:(s + 1) * buf] = xb * xb + yb * yb
            m6[5, s * buf:(s + 1) * buf] = -1.0
            for b in range(NBLK):
                m128[:, O_XSH + s * NBLK + b] = cxy[s][r0 + b * 128:r0 + b * 128 + 128, 0]
                m128[:, O_YSH + s * NBLK + b] = cxy[s][r0 + b * 128:r0 + b * 128 + 128, 1]
        # radial: owned slide (cores 0-2), mask row 0; pre-centered
        m128[:, O_RXY:O_RXY + 64] = cxy[core % NSLIDES][:, 0].reshape(128, 64)
        m128[:, O_RXY + 64:O_RXY + 128] = cxy[core % NSLIDES][:, 1].reshape(128, 64)
        if core < NSLIDES:
            m128[0, O_MASK + core * 20:O_MASK + (core + 1) * 20] = np.float32(1.0) / np.float32(N)
        in_maps.append({"m128": m128, "m6": m6})
    return in_maps


def _pick_win(coords_list):
    # win > 2048 would need a deeper PSUM chunking scheme; these whalf values
    # cover any remotely Gaussian-like input (the shipped inputs pass at 64)
    for whalf in (64, 192, 448, 960):
        ok = True
        for c in coords_list:
            xs = np.sort(c[:, 0])
            if (xs[whalf:] - xs[:-whalf]).min() < 0.01:
                ok = False
                break
        if ok:
            return 128 + 2 * whalf
    raise ValueError("no valid rank window (pathological input)")


def kernel(coords0, coords1, coords2, slide_labels=None, **_):
    coords_list = [np.ascontiguousarray(np.asarray(c, dtype=np.float32))
                   for c in (coords0, coords1, coords2)]
    assert coords_list[0].shape == (N, 2)
    win = _pick_win(coords_list)
    nc = _get_program(win)
    in_maps = _host_prep(coords_list, win)
    res = run_bass_kernel_spmd(nc, in_maps, core_ids=list(range(N_CORES)))
    val = np.float32(res.results[0]["out"][0, 0])
    return np.asarray(val, dtype=np.float32).reshape(())


# revision 13
# speedup vs baseline: 1.1824x; 1.0235x over previous
"""Trainium2 Bass kernel for nn_CrossSlideConsistencyLoss.

Computes, for 3 slides of 8192 2-D points each:
  - radial histogram (20 bins) of centered radii
  - |FFT|[0:5] of the mean-centered angular histogram (72 bins)
  - collision rate: fraction of points whose nearest neighbor is < 0.01 away
then the mean over descriptor components of the across-slide variance (ddof=1).

Strategy (8 NeuronCores, SPMD):
  - Host sorts each slide's points by x (pure permutation; every descriptor
    piece is permutation invariant). Any pair closer than 0.01 is then within
    whalf ranks of each other (validated at runtime), so the NxN cdist
    collapses to a banded window per 128-row block. One K=6 matmul per block
    computes z' = 2 x_i x_j + 2 y_i y_j - sq_j - sq_i + th directly (the
    per-row bias rides in matmul rows 4/5 with host-precomputed sq), so
    d2 < th  <=>  z' > 0 uniformly: ACT counts via Sign+accum, DVE via
    is_gt+accum, both with zero bias. Rows are sharded over the 8 cores.
  - Angular DFT: only |FFT| bins 1..4 of the angular histogram are needed
    (bin 0 of the mean-centered histogram is ~0); they equal direct sums of
    cos/sin(2 pi k aidx / 72) over points - computed shard-local on every
    core and summed by the final AllReduce. The floor/range-reduce helper
    chain runs on the (otherwise idle) gpsimd engine.
  - Radial histogram: one core per slide bins it whole (mask-gated). Radii
    are rescaled to g = rf2 * 400 / (rmax2 + 1e-8) so all 19 bin thresholds
    become the constants k^2 (bin 20 is the known total = 1.0 after
    normalization); the compare+count runs as two fat fp16 2x-mode ops
    mid-stream on DVE.
  - One 512B AllReduce combines [radial counts | DFT sums | collision
    counts]; every core then computes the final variance; core 0's scalar
    output is returned.

Emission order is engine-schedule-aware: per-engine queues execute in order,
so instructions are emitted in the order each engine should run them
(earliest-ready first), with a few explicit cross-engine ordering deps.
"""
import numpy as np

import concourse.bass as bass
import concourse.bacc as bacc
import concourse.bass_isa as bass_isa
import concourse.mybir as mybir
import concourse.tile as tile
from concourse.bass_utils import run_bass_kernel_spmd

F32 = mybir.dt.float32
F32R = mybir.dt.float32r
F16 = mybir.dt.float16
Alu = mybir.AluOpType
Act = mybir.ActivationFunctionType

N = 8192
N_CORES = 8
NSLIDES = 3
ROWS_PER_CORE = N // N_CORES          # 1024
NBLK = ROWS_PER_CORE // 128           # 8 blocks per core per slide
SH = NSLIDES * NBLK                   # 24 shard columns
TH = 1e-4                             # d^2 threshold (0.01^2)
PI = float(np.pi)
R2C = 12582912.0                      # 1.5 * 2^23: rne magic constant
NRB = 19                              # compared radial cum-bins (bin 20 = 1)

# collision block assignment: DVE handles the earliest-produced blocks of
# slide 0 (ready before the input DMAs for the shard chain land) plus the
# tail blocks of slides 1-2; ACT takes the rest (13 blocks). ACT signs cost
# 398+187 ns each (activation + accumulator read), DVE is_gt ~392, so DVE
# carries one block less than an even split despite its fatter side chain.
DVE_OF = {0: (0, 1, 2), 1: (4, 5, 6, 7), 2: (4, 5, 6, 7)}
N_EARLY = 3                           # (s0, b0..2) hoisted before the chain

# AllReduce vector layout ([1, 128] f32):
#   [0:63)   radial cumcounts, per slide: [C0=0, C1..C20] (21 cols x 3)
#   [63:87)  DFT sums: 12 cos (k-major, s-minor), then 12 sin
#   [87:90)  collision row counts per slide
V_DFT = 63
V_COLL = 87

# mega-input column layout for m128 [128, F1] (f32); slide coords arrive
# pre-centered from the host (distances in m6 use the raw coords)
#   xsh(24) | ysh(24) | thr24(24) | rxy(128) | mask row0 (64)
#   | kk2 fp16 pairs (19*64/2 = 608)
O_XSH = 0
O_YSH = O_XSH + SH
O_THR = O_YSH + SH
O_RXY = O_THR + SH
O_MASK = O_RXY + 128
O_KK2 = O_MASK + 64
F1 = O_KK2 + NRB * 32


def _bcast(ap, axis_len, at):
    """Insert a broadcast (stride-0) dim of length axis_len at free position
    `at` (0 = before the flattened free dim, 1 = after it)."""
    p, f = ap.shape[0], int(np.prod(ap.shape[1:]))
    if at == 0:
        return ap.rearrange("p (a b) -> p a b", a=1).to_broadcast([p, axis_len, f])
    return ap.rearrange("p (a b) -> p a b", b=1).to_broadcast([p, f, axis_len])


def build_program(win, collective=True):
    buf = ROWS_PER_CORE + win - 128   # rhs window buffer length per core/slide
    assert buf % 128 == 0

    # m6 [6, F2]: rhs buffers (x,y,x,y,sq,-1) | lhs (x,y,x,y,-1,sq-th)
    O_LHS = NSLIDES * buf
    F2 = O_LHS + NSLIDES * ROWS_PER_CORE

    nc = bacc.Bacc("TRN2", target_bir_lowering=False, debug=False, num_devices=N_CORES)
    i_m128 = nc.dram_tensor("m128", [128, F1], F32, kind="ExternalInput")
    i_m6 = nc.dram_tensor("m6", [6, F2], F32R, kind="ExternalInput")
    o_out = nc.dram_tensor("out", [1, 1], F32, kind="ExternalOutput")

    with tile.TileContext(nc) as tc:
        with (
            tc.tile_pool(name="cst", bufs=1) as cst,
            tc.tile_pool(name="scr", bufs=3) as scp,
            tc.tile_pool(name="psum",
                         bufs=max(1, 6 // max(1, win * 4 // 2048)),
                         space="PSUM") as pp,
            tc.tile_pool(name="psv", bufs=1, space="PSUM") as pv,
            tc.tile_pool(name="dram", bufs=1, space="DRAM") as dr,
        ):
            # ------------- input loads (3 DMAs: shard | m6 | rest) ---------
            big128 = cst.tile([128, F1], F32, tag="big128")
            nc.sync.dma_start(big128[:, 0:O_RXY], i_m128[:, 0:O_RXY])
            big6 = cst.tile([6, F2], F32R, tag="big6")
            nc.sync.dma_start(big6[:], i_m6[:])
            nc.sync.dma_start(big128[:, O_RXY:F1], i_m128[:, O_RXY:F1])


            rxy = big128[:, O_RXY:O_RXY + 128]
            xsh = big128[:, O_XSH:O_XSH + SH]
            ysh = big128[:, O_YSH:O_YSH + SH]
            maskR = big128[0:1, O_MASK:O_MASK + 64]
            thr24 = big128[:, O_THR:O_THR + SH]
            kk2 = big128[:, O_KK2:O_KK2 + NRB * 32].bitcast(F16)  # [128, 19*64]
            rhs = [big6[:, buf * s:buf * (s + 1)] for s in range(NSLIDES)]
            lhsT = big6[:, O_LHS:O_LHS + NSLIDES * ROWS_PER_CORE]

            ones128 = cst.tile([128, 1], F32, tag="ones128")
            nc.gpsimd.memset(ones128[:], 1.0)
            invN128 = cst.tile([128, 1], F32, tag="invN128")
            nc.gpsimd.memset(invN128[:], 1.0 / N)
            ones128h = cst.tile([128, 1], F16, tag="ones128h")
            nc.gpsimd.memset(ones128h[:], 1.0)
            vecS = cst.tile([1, 128], F32, tag="vecS")
            nc.gpsimd.memset(vecS[:], 0.0)
            nc.scalar.add_instruction(mybir.InstLoadActFuncSet(
                act_func_set_id=9, name=f"I-{nc.next_id()}", ins=[], outs=[]))

            acc = cst.tile([128, SH], F32, tag="acc")
            vecp = pv.tile([1, 64], F32, tag="vecp")
            sign_insts = []
            dve_cmp_insts = []

            def emit_block(s, b):
                col = s * NBLK + b
                zp = pp.tile([128, win], F32, tag="zp")
                for off in range(0, win, 512):  # moving free dim cap is 512
                    nc.tensor.matmul(
                        zp[:, off:off + min(512, win - off)],
                        lhsT[:, s * ROWS_PER_CORE + b * 128:
                             s * ROWS_PER_CORE + b * 128 + 128],
                        rhs[s][:, b * 128 + off:b * 128 + off + min(512, win - off)],
                    )
                if b in DVE_OF[s]:
                    sg = scp.tile([128, win], F32, tag="sgd")
                    dve_cmp_insts.append(nc.vector.tensor_scalar(
                        sg[:], zp[:], 0.0, None, Alu.is_gt,
                        Alu.add, accum_out=acc[:, col:col + 1],
                    ).ins)
                else:
                    sg = scp.tile([128, win], F32, tag="sg")
                    sign_insts.append(nc.scalar.activation(
                        sg[:], zp[:], Act.Sign, accum_out=acc[:, col:col + 1],
                    ).ins)

            # early collision blocks: DVE chews these while the input DMAs for
            # the shard chain are still in flight
            for b in range(N_EARLY):
                emit_block(0, b)

            # ---------------- shard chains (coords pre-centered) -----------
            dx = xsh
            dy = ysh
            r2 = scp.tile([128, SH], F32, tag="r2")
            nc.vector.tensor_tensor(r2[:], dx[:], dx[:], Alu.mult)
            yy = scp.tile([128, SH], F32, tag="yy")
            nc.vector.tensor_tensor(yy[:], dy[:], dy[:], Alu.mult)
            nc.vector.tensor_tensor(r2[:], r2[:], yy[:], Alu.add)
            I32 = mybir.dt.int32
            ih = scp.tile([128, SH], I32, tag="ih")
            nc.vector.tensor_scalar(ih[:], r2[:].bitcast(I32), 1, None, Alu.arith_shift_right)
            nc.vector.tensor_scalar(ih[:], ih[:], -1, 0x5F3759DF, Alu.mult, Alu.add)
            ny = scp.tile([128, SH], F32, tag="ny")
            nc.vector.tensor_copy(ny[:], ih[:].bitcast(F32))
            for _ in range(2):
                nt = scp.tile([128, SH], F32, tag="nt")
                nc.vector.tensor_tensor(nt[:], ny[:], ny[:], Alu.mult)
                nc.vector.scalar_tensor_tensor(nt[:], nt[:], -0.5, r2[:], Alu.mult, Alu.mult)
                nc.vector.scalar_tensor_tensor(ny[:], nt[:], 1.5, ny[:], Alu.add, Alu.mult)
            rr = scp.tile([128, SH], F32, tag="rr")
            nc.vector.tensor_tensor(rr[:], r2[:], ny[:], Alu.mult)

            # ---------------- radial + angle prep (gpsimd) -----------------
            dxf = rxy[:, 0:64]
            dyf = rxy[:, 64:128]
            rf2 = scp.tile([128, 64], F32, tag="rf2")
            nc.gpsimd.tensor_tensor(rf2[:], dxf, dxf, Alu.mult)
            yyf = scp.tile([128, 64], F32, tag="yyf")
            nc.gpsimd.tensor_tensor(yyf[:], dyf, dyf, Alu.mult)
            nc.gpsimd.tensor_tensor(rf2[:], rf2[:], yyf[:], Alu.add)
            neg = scp.tile([128, SH], F32, tag="neg")
            nc.gpsimd.tensor_scalar(neg[:], dx, 0.0, None, Alu.is_lt)
            c1 = scp.tile([128, SH], F32, tag="c1")
            nc.gpsimd.tensor_scalar(c1[:], neg[:], -2.0, 1.0, Alu.mult, Alu.add)
            sy36 = scp.tile([128, SH], F32, tag="sy36")
            nc.gpsimd.tensor_scalar(sy36[:], dy, 0.0, 72.0, Alu.is_ge, Alu.mult)
            nc.gpsimd.tensor_scalar(sy36[:], sy36[:], 36.0, None, Alu.subtract)
            pn36p = scp.tile([128, SH], F32, tag="pn36p")
            nc.gpsimd.tensor_tensor(pn36p[:], neg[:], sy36[:], Alu.mult)
            nc.gpsimd.tensor_scalar(pn36p[:], pn36p[:], 36.0, None, Alu.add)

            # ---------------- DVE mid chain --------------------------------
            rmx = scp.tile([128, 1], F32, tag="rmx")
            nc.vector.tensor_reduce(rmx[:], rf2[:], mybir.AxisListType.X, Alu.max)
            adx = scp.tile([128, SH], F32, tag="adx")
            adx_bi = nc.scalar.activation(adx[:], dx, Act.Abs)
            den = scp.tile([128, SH], F32, tag="den")
            nc.vector.scalar_tensor_tensor(den[:], rr[:], 1e-38, adx[:], Alu.add, Alu.add)
            rden = scp.tile([128, SH], F32, tag="rden")
            nc.vector.reciprocal(rden[:], den[:])
            qt = scp.tile([128, SH], F32, tag="qt")
            nc.vector.tensor_tensor(qt[:], dy, rden[:], Alu.mult)
            rmxB = scp.tile([128, 1], F32, tag="rmxB")
            nc.gpsimd.partition_all_reduce(rmxB[:], rmx[:], 128, bass_isa.ReduceOp.max)
            rm1 = scp.tile([128, 1], F32, tag="rm1")
            nc.vector.tensor_scalar(rm1[:], rmxB[:], 1e-8, None, Alu.add)
            rcp = scp.tile([128, 1], F32, tag="rcp")
            nc.vector.reciprocal(rcp[:], rm1[:])
            # g = rf2 * 400 / (rmax2 + 1e-8); bins are then g < k^2 (const)
            g16 = scp.tile([128, 64], F16, tag="g16")
            nc.vector.tensor_scalar(g16[:], rf2[:], rcp[:, 0:1], 400.0, Alu.mult, Alu.mult)

            at = scp.tile([128, SH], F32, tag="at")
            scv = scp.tile([128, 8 * SH], F32, tag="scv")
            mb = scp.tile([128, 8 * SH], F32, tag="mb")
            vb = scp.tile([128, 8 * SH], F32, tag="vb")

            def emit_dft_mid():
                # arctan (ACT) -> t1/ut (DVE) -> floor+range-reduce (Pool);
                # the Pool chain runs concurrently with the collision stream
                at_bi = nc.scalar.activation(at[:], qt[:], Act.Arctan)
                bass._add_dep_helper(at_bi.ins, sign_insts[-1], False,
                                     "order: arctan after 5 signs")
                t1 = scp.tile([128, SH], F32, tag="t1")
                nc.vector.tensor_tensor(t1[:], at[:], c1[:], Alu.mult)
                ut = scp.tile([128, SH], F32, tag="ut")
                nc.vector.scalar_tensor_tensor(ut[:], t1[:], 72.0 / PI, pn36p[:],
                                               Alu.mult, Alu.add)
                # floor + range-reduce: rv/vb/mb scalar stages on Pool (the
                # only tensor-tensor/STT forms the HW Pool engine accepts are
                # add/mult, so the compares and subtracts stay on DVE)
                rv = scp.tile([128, SH], F32, tag="rv")
                nc.gpsimd.tensor_scalar(rv[:], ut[:], R2C, R2C, Alu.add, Alu.subtract)
                cmp = scp.tile([128, SH], F32, tag="cmp")
                nc.vector.tensor_tensor(cmp[:], rv[:], ut[:], Alu.is_gt)
                nfl = scp.tile([128, SH], F32, tag="nfl")
                nc.vector.tensor_tensor(nfl[:], cmp[:], rv[:], Alu.subtract)
                ka = scp.tile([128, 4 * SH], F32, tag="ka")
                for k in range(1, 5):
                    nc.gpsimd.tensor_scalar(ka[:, (k - 1) * SH:k * SH], nfl[:],
                                            -71.0, float(k), Alu.max, Alu.mult)
                nc.gpsimd.tensor_scalar(vb[:, 0:4 * SH], ka[:], 1.0 / 72.0, None, Alu.mult)
                nc.gpsimd.tensor_scalar(vb[:, 4 * SH:8 * SH], ka[:], 1.0 / 72.0, 0.25,
                                        Alu.mult, Alu.add)
                nc.gpsimd.tensor_scalar(mb[:], vb[:], R2C, R2C, Alu.add, Alu.subtract)

            # ---------------- remaining collision blocks -------------------
            # ACT stream: signs with arctan/sin spliced in at readiness points
            nsign = 0
            sin_bi = None
            for s in range(NSLIDES):
                for b in range(NBLK):
                    if s == 0 and b < N_EARLY:
                        continue
                    emit_block(s, b)
                    if b not in DVE_OF[s]:
                        nsign += 1
                        if nsign == 5:
                            emit_dft_mid()
                        elif nsign == 12:
                            nc.vector.tensor_tensor(mb[:], vb[:], mb[:], Alu.subtract)
                            sin_bi = nc.scalar.activation(scv[:], mb[:], Act.Sin,
                                                          scale=2.0 * PI)
                            bass._add_dep_helper(sin_bi.ins, sign_insts[-1], False,
                                                 "order: sin after 12 signs")

            # ---------------- radial histogram (fp16, mid-stream) ----------
            ct = scp.tile([128, NRB * 64], F16, tag="ct")
            nc.vector.tensor_tensor(
                ct[:].rearrange("p (k f) -> p k f", k=NRB),
                _bcast(g16[:], NRB, 0),
                kk2.rearrange("p (k f) -> p k f", k=NRB), Alu.is_lt,
            )
            cr = scp.tile([128, NRB], F16, tag="cr")
            with nc.allow_low_precision(reason="fp16 radial counts <= 64 are exact"):
                nc.vector.tensor_reduce(
                    cr[:], ct[:].rearrange("p (k f) -> p k f", k=NRB),
                    mybir.AxisListType.X, Alu.add,
                )
            nc.tensor.matmul(vecp[0:1, 0:NRB], ones128h[:], cr[:])

            # ---------------- trig reduces + DFT matmul --------------------
            sincos = cst.tile([128, 24], F32, tag="sincos")
            nc.vector.tensor_reduce(
                sincos[:, 12:24].rearrange("p (k s) -> p k s", k=4),
                scv[:, 0:4 * SH].rearrange("p (k s b) -> p k s b", k=4, s=NSLIDES),
                mybir.AxisListType.X, Alu.add,
            )
            nc.vector.tensor_reduce(
                sincos[:, 0:12].rearrange("p (k s) -> p k s", k=4),
                scv[:, 4 * SH:8 * SH].rearrange("p (k s b) -> p k s b", k=4, s=NSLIDES),
                mybir.AxisListType.X, Alu.add,
            )
            nc.tensor.matmul(vecp[0:1, 20:44], ones128[:], sincos[:])

            # ---------------- collision indicators -------------------------
            # ACT cols: sumsign > 3-win <=> count >= 2; DVE cols: count > 1.5
            # (per-column thresholds ride in from the host via thr24)
            ind = scp.tile([128, SH], F32, tag="ind")
            nc.vector.tensor_tensor(ind[:], acc[:], thr24, Alu.is_gt)
            indR = scp.tile([128, NSLIDES], F32, tag="indR")
            nc.vector.tensor_reduce(
                indR[:], ind[:].rearrange("p (s b) -> p s b", s=NSLIDES),
                mybir.AxisListType.X, Alu.add,
            )
            nc.tensor.matmul(vecp[0:1, 44:47], invN128[:], indR[:])

            # ---------------- assemble + AllReduce -------------------------
            radview = vecS[0:1, 0:63].rearrange(
                "p (s k) -> p s k", s=NSLIDES, k=21
            )[:, :, 1:1 + NRB]
            nc.vector.tensor_tensor(
                radview,
                vecp[0:1, 0:NRB].rearrange("p (o k) -> p o k", o=1)
                .to_broadcast([1, NSLIDES, NRB]),
                maskR[0:1, 0:60].rearrange("p (s k) -> p s k", s=NSLIDES)[:, :, 0:NRB],
                Alu.mult,
            )
            # C20 = 1.0 for the owner core (counts < rmax2+1e-8 is all of them)
            c20v = vecS[0:1, 20:83].rearrange("p (s r) -> p s r", s=NSLIDES)[:, :, 0:1]
            m20v = maskR[0:1, 0:60].rearrange("p (s r) -> p s r", s=NSLIDES)[:, :, 0:1]
            nc.vector.tensor_scalar(c20v, m20v, float(N), None, Alu.mult)
            nc.vector.tensor_copy(vecS[0:1, V_DFT:V_DFT + 27], vecp[0:1, 20:47])

            # preload the sqrt act-table before the collective so the post
            # stage pays no table load in the tail
            s2t = scp.tile([1, 1], F32, tag="s2t")
            nc.vector.tensor_tensor(s2t[:], sincos[0:1, 0:1], sincos[0:1, 0:1], Alu.mult)
            dum = scp.tile([1, 1], F32, tag="dum")
            dum_bi = nc.scalar.activation(dum[:], s2t[:], Act.Sqrt)
            bass._add_dep_helper(dum_bi.ins, sign_insts[-1], False,
                                 "order: sqrt table preload after collision signs")
            bass._add_dep_helper(dum_bi.ins, sin_bi.ins, False,
                                 "order: sqrt table preload after sin")

            ccin = dr.tile([1, 128], F32)
            ccout = dr.tile([1, 128], F32, addr_space="Shared")
            nc.sync.dma_start(ccin[:], vecS[:])
            if collective:
                nc.gpsimd.collective_compute(
                    "AllReduce", Alu.add,
                    replica_groups=[list(range(N_CORES))],
                    ins=[ccin.opt()], outs=[ccout.opt()],
                )
            else:
                nc.sync.dma_start(ccout[:], ccin[:])
            vecR = cst.tile([1, 128], F32, tag="vecR")
            nc.sync.dma_start(vecR[:], ccout[:])

            # ---------------- descriptors + variance -----------------------
            SC = cst.tile([1, 76], F32, tag="SC")
            # hist fractions (radial counts were pre-scaled by 1/N via the mask);
            # vec radial layout is s-major (s:3, j:21); output dims follow (s, j).
            rad63 = vecR[0:1, 0:63].rearrange("p (s j) -> p s j", s=NSLIDES)
            difv = SC[0:1, 0:60].rearrange("p (j s) -> p s j", j=20)
            nc.vector.tensor_tensor(difv, rad63[:, :, 1:21], rad63[:, :, 0:20], Alu.subtract)
            # power spectrum k=1..4: sqrt(cos^2 + sin^2)
            t24 = scp.tile([1, 24], F32, tag="t24")
            nc.vector.tensor_tensor(t24[:], vecR[0:1, V_DFT:V_DFT + 24],
                                    vecR[0:1, V_DFT:V_DFT + 24], Alu.mult)
            nc.vector.tensor_tensor(SC[0:1, 60:72], t24[0:1, 0:12], t24[0:1, 12:24], Alu.add)
            nc.scalar.activation(SC[0:1, 60:72], SC[0:1, 60:72], Act.Sqrt)
            nc.vector.tensor_copy(SC[0:1, 72:75], vecR[0:1, V_COLL:V_COLL + 3])
            # variance over slides (ddof=1), mean over 26 components
            m25 = scp.tile([1, 25], F32, tag="m25")
            nc.vector.tensor_reduce(
                m25[:], SC[0:1, 0:75].rearrange("p (c s) -> p c s", c=25),
                mybir.AxisListType.X, Alu.add,
            )
            # dev = mean - x (sign-flipped; squared next)
            dev = scp.tile([1, 75], F32, tag="dev")
            nc.vector.scalar_tensor_tensor(
                dev[:].rearrange("p (c s) -> p c s", c=25),
                _bcast(m25[:], NSLIDES, 1), 1.0 / NSLIDES,
                SC[0:1, 0:75].rearrange("p (c s) -> p c s", c=25),
                Alu.mult, Alu.subtract,
            )
            nc.vector.tensor_tensor(dev[:], dev[:], dev[:], Alu.mult)
            tot = scp.tile([1, 1], F32, tag="tot")
            nc.vector.tensor_reduce(
                tot[:], dev[:].rearrange("p (c s) -> p c s", c=25),
                mybir.AxisListType.XY, Alu.add,
            )
            outS = scp.tile([1, 1], F32, tag="outS")
            nc.vector.tensor_scalar(outS[:], tot[:], 1.0 / (2.0 * 26.0), None, Alu.mult)
            nc.sync.dma_start(o_out[:], outS[:])

    nc.compile()
    return nc


_PROG_CACHE = {}


def _get_program(win):
    if win not in _PROG_CACHE:
        _PROG_CACHE[win] = build_program(win)
    return _PROG_CACHE[win]


def _host_prep(coords_list, win):
    whalf = (win - 128) // 2
    buf = ROWS_PER_CORE + win - 128
    SENT_X = np.float32(1e6)

    O_LHS = NSLIDES * buf
    F2 = O_LHS + NSLIDES * ROWS_PER_CORE

    sxy = []
    cxy = []
    for c in coords_list:
        order = np.argsort(c[:, 0], kind="stable")
        s = np.ascontiguousarray(c[order])
        sxy.append(s)
        # pre-centered copy for the histogram/DFT paths (f32 center, matching
        # the reference's coords.mean(axis=0) up to summation order)
        center = c.astype(np.float64).mean(axis=0).astype(np.float32)
        cxy.append((s - center).astype(np.float32))

    base128 = np.zeros((128, F1), np.float32)
    # per-column collision indicator thresholds (ACT sign-sum vs DVE count)
    thr = np.empty(SH, np.float32)
    for s in range(NSLIDES):
        for b in range(NBLK):
            thr[s * NBLK + b] = np.float32(1.5 if b in DVE_OF[s] else 3.0 - win)
    base128[:, O_THR:O_THR + SH] = thr
    # radial bin thresholds k^2 (fp16 exact), bin-major, packed into f32 pairs
    kk2 = np.repeat((np.arange(1, NRB + 1, dtype=np.float16)) ** 2, 64)
    base128[:, O_KK2:O_KK2 + NRB * 32] = kk2.view(np.float32)[None, :]

    in_maps = []
    for core in range(N_CORES):
        r0 = core * ROWS_PER_CORE
        m128 = base128.copy()
        m6 = np.zeros((6, F2), np.float32)
        for s in range(NSLIDES):
            xs, ys = sxy[s][:, 0], sxy[s][:, 1]
            sl = slice(O_LHS + s * ROWS_PER_CORE, O_LHS + (s + 1) * ROWS_PER_CORE)
            xr = xs[r0:r0 + ROWS_PER_CORE]
            yr = ys[r0:r0 + ROWS_PER_CORE]
            m6[0, sl] = xr
            m6[1, sl] = yr
            m6[2, sl] = xr
            m6[3, sl] = yr
            m6[4, sl] = -1.0
            m6[5, sl] = (xr * xr + yr * yr) - np.float32(TH)
            # window buffer [r0-whalf, r0+1024+whalf) with sentinel padding
            xb = np.full(buf, SENT_X, np.float32)
            yb = np.zeros(buf, np.float32)
            g0 = r0 - whalf
            lo, hi = max(g0, 0), min(g0 + buf, N)
            xb[lo - g0:hi - g0] = xs[lo:hi]
            yb[lo - g0:hi - g0] = ys[lo:hi]
            m6[0, s * buf:(s + 1) * buf] = xb
            m6[1, s * buf:(s + 1) * buf] = yb
            m6[2, s * buf:(s + 1) * buf] = xb
            m6[3, s * buf:(s + 1) * buf] = yb
            m6[4, s * buf:(s + 1) * buf] = xb * xb + yb * yb
            m6[5, s * buf:(s + 1) * buf] = -1.0
            for b in range(NBLK):
                m128[:, O_XSH + s * NBLK + b] = cxy[s][r0 + b * 128:r0 + b * 128 + 128, 0]
                m128[:, O_YSH + s * NBLK + b] = cxy[s][r0 + b * 128:r0 + b * 128 + 128, 1]
        # radial: owned slide (cores 0-2), mask row 0; pre-centered
        m128[:, O_RXY:O_RXY + 64] = cxy[core % NSLIDES][:, 0].reshape(128, 64)
        m128[:, O_RXY + 64:O_RXY + 128] = cxy[core % NSLIDES][:, 1].reshape(128, 64)
        if core < NSLIDES:
            m128[0, O_MASK + core * 20:O_MASK + (core + 1) * 20] = np.float32(1.0) / np.float32(N)
        in_maps.append({"m128": m128, "m6": m6})
    return in_maps


def _pick_win(coords_list):
    # win > 2048 would need a deeper PSUM chunking scheme; these whalf values
    # cover any remotely Gaussian-like input (the shipped inputs pass at 64)
    for whalf in (64, 192, 448, 960):
        ok = True
        for c in coords_list:
            xs = np.sort(c[:, 0])
            if (xs[whalf:] - xs[:-whalf]).min() < 0.01:
                ok = False
                break
        if ok:
            return 128 + 2 * whalf
    raise ValueError("no valid rank window (pathological input)")


def kernel(coords0, coords1, coords2, slide_labels=None, **_):
    coords_list = [np.ascontiguousarray(np.asarray(c, dtype=np.float32))
                   for c in (coords0, coords1, coords2)]
    assert coords_list[0].shape == (N, 2)
    win = _pick_win(coords_list)
    nc = _get_program(win)
    in_maps = _host_prep(coords_list, win)
    res = run_bass_kernel_spmd(nc, in_maps, core_ids=list(range(N_CORES)))
    val = np.float32(res.results[0]["out"][0, 0])
    return np.asarray(val, dtype=np.float32).reshape(())
